# revision 16
# baseline (speedup 1.0000x reference)
"""Trainium2 Bass kernel for a 3-sublayer decoder block (nn_DecoderLayer).

Reference computation (B=2, S=2048, D=1024, H=16, DK=64, FF=4096, fp32):
  sa = causal_mha(x, x)          ; x1 = seqnorm(sa + x)
  ca = mha(x1, enc)              ; x2 = seqnorm(ca + x1)
  ffn = relu(x2 @ W1 + b1) @ W2 + b2 ; out = seqnorm(ffn + x2)
seqnorm normalizes over the SEQUENCE dim and divides by the unbiased VARIANCE
(reference quirk); attention has no output projection.

Sharding (8 cores, one replica group): every core processes BOTH batch
elements; heads split 2-per-core (a 128-wide channel slice of every sublayer
output per batch stays fully local, incl. seqnorm); FF hidden split
512-per-core (Megatron column/row). Collectives: AllGather(x1) and
AllGather(x2) per BATCH element (so AG(b0) hides under attention compute of
b1), ReduceScatter of FFN partials per (batch, seq-half) (so only the last
quarter-ish is tail-exposed).

v2 vs the fp32r baseline (1.44 ms):
 * fp16 operands end-to-end (host-cast): halves HBM+collective bytes, enables
   FWL fast weight loads (fp32 LDWEIGHTS was 423us of serialized PE time),
   and drops PE power (the fp32 version sat at K=4/8 = 1.2 GHz throttle).
 * per-batch collective pipelining (above) removes the ~310us of dead zones.
 * big streaming loads on the sync (HWDGE) queue, collective bounce stores as
   casting SWDGE DMAs on the gpsimd queue - so enc/x1f/x2f prefetch is never
   stuck behind an AllGather in the same in-order queue.
 * softmax denominator reciprocal via reciprocal_approx_fast (custom DVE op,
   ~5x faster than the iterative divide: that was 107us of DVE time).
 * PSUM: score tiles double-buffered (2x[128,1024]) so scores(c+1) overlaps
   exp(c) on ACT; PV accumulates both heads into one [128,1024] bank pair
   drained in a single copy.
All activations transposed on-chip ([d, s]); E^T = exp(K @ Q^T) computed
directly; softmax denominators from ones-columns appended to V.
"""

import os
import sys

import numpy as np

for _p in ("/opt/trn_rl_repo", "/root/.axon_site/_ro/trn_rl_repo"):
    if _p not in sys.path and os.path.isdir(_p):
        sys.path.append(_p)

import concourse.bass as bass
import concourse.mybir as mybir
import concourse.tile as tile
from concourse import bacc
from concourse.bass import ts
from concourse.bass_utils import run_bass_kernel_spmd

F32 = mybir.dt.float32
F16 = mybir.dt.float16
AF = mybir.ActivationFunctionType
ALU = mybir.AluOpType

B, S, D, H = 2, 2048, 1024, 16
DK = D // H            # 64
FF = 4 * D             # 4096
NCORES = 8
HL = H // NCORES       # 2 heads per core
DL = DK * HL           # 128 channels per core
FFL = FF // NCORES     # 512 ff dims per core
KC = D // 128          # 8 contraction chunks of the full model dim
FCL = FFL // 128       # 4 local ff chunks
ST = S // 512          # 4 sequence tiles of 512
SC = S // 128          # 16 sequence chunks of 128
SH = S // 2            # sequence half
SCALE = 1.0 / np.sqrt(DK)
VARF = (S - 1) / S     # unbiased-variance factor applied to 1/var_pop
VW = DK + 1            # 65: per-head stationary width in vO (v + ones col)

RG = [[0, 1, 2, 3, 4, 5, 6, 7]]

_CACHED_NC = None


def _build():
    nc = bacc.Bacc("TRN2", target_bir_lowering=False, debug=False,
                   num_devices=NCORES)

    # ---- per-core external inputs (fp16 except small fp32 biases) ----
    xTd = nc.dram_tensor("xTd", [128, B, KC, S], F16, kind="ExternalInput")
    encd = nc.dram_tensor("encd", [128, B, KC, S], F16, kind="ExternalInput")
    resd = nc.dram_tensor("resd", [128, B, S], F16, kind="ExternalInput")
    wq1d = nc.dram_tensor("wq1d", [128, KC, DL], F16, kind="ExternalInput")
    wk1d = nc.dram_tensor("wk1d", [128, KC, DL], F16, kind="ExternalInput")
    wv1d = nc.dram_tensor("wv1d", [128, KC, DL], F16, kind="ExternalInput")
    wq2d = nc.dram_tensor("wq2d", [128, KC, DL], F16, kind="ExternalInput")
    wk2d = nc.dram_tensor("wk2d", [128, KC, DL], F16, kind="ExternalInput")
    wv2d = nc.dram_tensor("wv2d", [128, KC, DL], F16, kind="ExternalInput")
    w1d = nc.dram_tensor("w1d", [128, KC, FFL], F16, kind="ExternalInput")
    w2d = nc.dram_tensor("w2d", [128, FCL, D], F16, kind="ExternalInput")
    bqkd = nc.dram_tensor("bqkd", [128, 4], F32, kind="ExternalInput")
    # bv values per (head, dk); rows 64:128 duplicate rows 0:64 so the h1
    # stt can use a partition-64-based scalar AP
    bvd = nc.dram_tensor("bvd", [128, 2 * HL], F32, kind="ExternalInput")
    b1d = nc.dram_tensor("b1d", [128, FCL], F32, kind="ExternalInput")
    b2d = nc.dram_tensor("b2d", [128, 1], F32, kind="ExternalInput")

    outT = nc.dram_tensor("outT", [B * DL, S], F32, kind="ExternalOutput")
    DBG = bool(os.environ.get("BASSDBG"))
    if DBG:
        dbg_x1 = nc.dram_tensor("dbg_x1", [B * DL, S], F32,
                                kind="ExternalOutput")
        dbg_x2 = nc.dram_tensor("dbg_x2", [B * DL, S], F32,
                                kind="ExternalOutput")
        dbg_k1 = nc.dram_tensor("dbg_k1", [128, B, S], F16,
                                kind="ExternalOutput")
        dbg_q1 = nc.dram_tensor("dbg_q1", [128, B, ST, 512], F16,
                                kind="ExternalOutput")
        dbg_v1 = nc.dram_tensor("dbg_v1", [128, B, SC, 2 * VW], F16,
                                kind="ExternalOutput")
        dbg_pre = nc.dram_tensor("dbg_pre", [128, B, S], F32,
                                 kind="ExternalOutput")

    def bview(t):   # [B*128, s] -> [p, b, s]
        return t[:].rearrange("(b p) s -> p b s", p=128)

    with tile.TileContext(nc) as tc:
        import contextlib
        ctx = contextlib.ExitStack()
        with ctx:
            sb = ctx.enter_context(tc.tile_pool(name="sb", bufs=1))
            dram = ctx.enter_context(tc.tile_pool(name="dr", bufs=1,
                                                  space="DRAM"))
            pp = ctx.enter_context(tc.tile_pool(name="pp", bufs=2,
                                                space="PSUM"))
            pe = ctx.enter_context(tc.tile_pool(name="pe", bufs=2,
                                                space="PSUM"))
            pz = ctx.enter_context(tc.tile_pool(name="pz", bufs=1,
                                                space="PSUM"))

            # ---- collective bounce buffers (per batch element) ----
            x1b = [dram.tile([DL, S], F16, tag=f"x1b{b}", name=f"x1b{b}")
                   for b in range(B)]
            x1f = [dram.tile([NCORES, DL, S], F16, tag=f"x1f{b}",
                             name=f"x1f{b}", addr_space="Shared")
                   for b in range(B)]
            x2b = [dram.tile([DL, S], F16, tag=f"x2b{b}", name=f"x2b{b}")
                   for b in range(B)]
            x2f = [dram.tile([NCORES, DL, S], F16, tag=f"x2f{b}",
                             name=f"x2f{b}", addr_space="Shared")
                   for b in range(B)]
            # RS per (b, seq-half)
            rsi = [[dram.tile([NCORES, DL, SH], F16, tag=f"rsi{b}{h}",
                              name=f"rsi{b}{h}") for h in range(2)]
                   for b in range(B)]
            rso = [[dram.tile([DL, SH], F16, tag=f"rso{b}{h}",
                              name=f"rso{b}{h}")
                    for h in range(2)] for b in range(B)]

            # ---- small persistent tiles ----
            bqk_sb = sb.tile([128, 4], F32, tag="bias", bufs=1)
            nc.sync.dma_start(out=bqk_sb, in_=bqkd[:])
            bv_sb = sb.tile([128, 2 * HL], F32, tag="bias2", bufs=1)
            nc.sync.dma_start(out=bv_sb, in_=bvd[:])
            b1_sb = sb.tile([128, FCL], F32, tag="bias3", bufs=1)
            nc.sync.dma_start(out=b1_sb, in_=b1d[:])
            b2_sb = sb.tile([128, 1], F32, tag="bias4", bufs=1)
            nc.sync.dma_start(out=b2_sb, in_=b2d[:])

            # identity for PE transposes (fp16)
            id16 = sb.tile([128, 128], F16, tag="id16", bufs=1)
            nc.vector.memset(id16, 1.0)
            nc.gpsimd.affine_select(out=id16, in_=id16,
                                    compare_op=ALU.is_equal, fill=0.0,
                                    base=0, channel_multiplier=-1,
                                    pattern=[[1, 128]])

            # ---- weights (all loaded up front; ~3 MB fp16) ----
            def load_w(dram_t, name, cols, tag):
                w = sb.tile([128, KC, cols], F16, tag=tag, bufs=1, name=name)
                nc.sync.dma_start(out=w, in_=dram_t[:])
                return w

            wq1 = load_w(wq1d, "wq1", DL, "wqkv")
            wk1 = load_w(wk1d, "wk1", DL, "wqkv2")
            wv1 = load_w(wv1d, "wv1", DL, "wqkv3")
            wq2 = load_w(wq2d, "wq2", DL, "wqkv4")
            wk2 = load_w(wk2d, "wk2", DL, "wqkv5")
            wv2 = load_w(wv2d, "wv2", DL, "wqkv6")
            w1 = sb.tile([128, KC, FFL], F16, tag="w1", bufs=1, name="w1")
            nc.sync.dma_start(out=w1, in_=w1d[:])
            w2 = sb.tile([128, FCL, D], F16, tag="w2", bufs=1, name="w2")
            nc.sync.dma_start(out=w2, in_=w2d[:])

            resid = sb.tile([128, B, S], F16, tag="res", bufs=1, name="resid")
            nc.sync.dma_start(out=resid, in_=resd[:])

            # ---- persistent activation tiles ----
            kT1 = sb.tile([128, B, S], F16, tag="kT1", bufs=1, name="kT1")
            kT2 = sb.tile([128, B, S], F16, tag="kT2", bufs=1, name="kT2")
            # vO layout: [h0 ch(64) | ones | h1 ch(64) | ones] => stationary
            # for head h is the contiguous [128, 65] slice at 65*h.
            vO1 = sb.tile([128, B, SC, 2 * VW], F16, tag="vO1", bufs=1,
                          name="vO1")
            vO2 = sb.tile([128, B, SC, 2 * VW], F16, tag="vO2", bufs=1,
                          name="vO2")
            for vO in (vO1, vO2):
                nc.vector.memset(vO[:, :, :, DK:DK + 1], 1.0)
                nc.vector.memset(vO[:, :, :, DK + VW:DK + VW + 1], 1.0)
            qt1 = sb.tile([128, B, ST, 512], F16, tag="qt", bufs=2,
                          name="qt1")
            qt2 = sb.tile([128, B, ST, 512], F16, tag="qt", bufs=2,
                          name="qt2")
            x1 = sb.tile([128, B, S], F32, tag="xl", bufs=2, name="x1")
            x2 = sb.tile([128, B, S], F32, tag="xl", bufs=2, name="x2")

            def proj128(xs, w, bias_col, out_ap):
                """One [128, 512] projection: out = W.T @ x + bias (ACT
                drain psum->sbuf fp16 with per-channel bias)."""
                ps = pp.tile([128, 512], F32, tag="pp", name="ps")
                for k in range(KC):
                    nc.tensor.matmul(ps, w[:, k, :], xs[:, k, :],
                                     start=(k == 0), stop=(k == KC - 1))
                nc.scalar.activation(out=out_ap, in_=ps, func=AF.Identity,
                                     bias=bqk_sb[:, bias_col:bias_col + 1],
                                     scale=1.0)

            def qkv_tile(xsrc, b, t, wq, wk, wv, kT, vO, qt, qcol, kcol,
                         bv_off):
                """One (b, 512-seq-tile): load x chunk-slices, q/k transposed
                projections, v projected then flipped back via PE transposes
                into vO (ones columns persist from the initial memset)."""
                xs = sb.tile([128, KC, 512], F16, tag="xs", bufs=4,
                             name="xs")
                nc.sync.dma_start(out=xs, in_=xsrc[:, b, :, ts(t, 512)])
                if wq is not None:
                    proj128(xs, wq, qcol, qt[:, b, t, :])
                proj128(xs, wk, kcol, kT[:, b, ts(t, 512)])
                # vT [128(2h*dk), 512] (+bias) then transpose per 128-block
                ps = pp.tile([128, 512], F32, tag="pp", name="ps")
                for k in range(KC):
                    nc.tensor.matmul(ps, wv[:, k, :], xs[:, k, :],
                                     start=(k == 0), stop=(k == KC - 1))
                # no bias here: bv is added after softmax-normalize (rows of
                # P sum to 1, so PV(v)+bv == P(v+bv) normalized)
                vt = sb.tile([128, 512], F16, tag="vt", bufs=2, name="vt")
                nc.scalar.activation(out=vt, in_=ps, func=AF.Copy)
                tp = pp.tile([128, 512], F16, tag="pp", name="tp")
                for sc in range(4):
                    nc.tensor.transpose(tp[:, ts(sc, 128)],
                                        vt[:, ts(sc, 128)], id16)
                # one strided copy: [sc, h, dk] -> vO cols {0:64, 65:129}
                nc.vector.tensor_copy(
                    out=vO[:, b, 4 * t:4 * t + 4, :]
                        .rearrange("p c (h w) -> p c h w", h=2)[:, :, :, 0:DK],
                    in_=tp[:, :].rearrange("p (c h w) -> p c h w", c=4, h=2))

            def attn_tile(b, t, qt, kT, vO, xout, resid_ap, bv_off, causal):
                """One (b, sq-tile): per sk-chunk, E^T for both heads in one
                2-bank psum (disjoint PE row groups -> concurrent), exp on
                ACT (fp16 out), causal mask on gpsimd, PV for both heads into
                one 2-bank psum; then one drain + approx-reciprocal denom +
                normalize + bias + residual into xout."""
                nchunks = (4 * t + 4) if causal else SC
                zps = pz.tile([128, 1024], F32, tag="pz", name="zps")
                for c in range(nchunks):
                    eps = pe.tile([128, 1024], F32, tag="pe", name="eps")
                    for h in range(HL):
                        hb = h * 64
                        nc.tensor.matmul(eps[:, ts(h, 512)],
                                         kT[hb:hb + 64, b, ts(c, 128)],
                                         qt[hb:hb + 64, b, t, :],
                                         start=True, stop=True)
                    et = sb.tile([128, 1024], F16, tag="E", bufs=3,
                                 name="et")
                    nc.scalar.activation(out=et, in_=eps, func=AF.Exp,
                                         scale=float(SCALE))
                    if causal and c >= 4 * t:
                        j = c - 4 * t
                        for h in range(HL):
                            nc.gpsimd.affine_select(
                                out=et[:, ts(h, 512)],
                                in_=et[:, ts(h, 512)],
                                compare_op=ALU.is_ge,
                                fill=0.0, base=-(j * 128),
                                channel_multiplier=-1,
                                pattern=[[1, 512]])
                    for h in range(HL):
                        nc.tensor.matmul(
                            zps[0:VW, ts(h, 512)],
                            vO[:, b, c, VW * h:VW * h + VW],
                            et[:, ts(h, 512)],
                            start=(c == 0), stop=(c == nchunks - 1))
                # drain [65, 1024] once; row 64 = softmax denominators
                zsb = sb.tile([VW, 1024], F32, tag="zsb", bufs=2, name="zsb")
                nc.vector.tensor_copy(out=zsb, in_=zps[0:VW, :])
                dr = sb.tile([1, 1024], F32, tag="dr", bufs=2, name="dr")
                nc.vector.tensor_copy(out=dr, in_=zsb[DK:DK + 1, :])
                rb = sb.tile([64, 1024], F32, tag="rb", bufs=2, name="rb")
                nc.gpsimd.partition_broadcast(out_ap=rb, in_ap=dr)
                nc.vector.reciprocal_approx_fast(out=rb, in_=rb)
                zn = sb.tile([64, 1024], F32, tag="zn", bufs=2, name="zn")
                nc.vector.tensor_mul(zn, zsb[0:DK, :], rb)
                # h0 writes in place; h1 needs a partition shift (stt operands
                # must share a start partition; only copies may shift)
                nc.vector.scalar_tensor_tensor(
                    out=xout[0:64, b, ts(t, 512)],
                    in0=zn[:, 0:512],
                    scalar=bv_sb[0:64, bv_off:bv_off + 1], op0=ALU.add,
                    in1=resid_ap[0:64, b, ts(t, 512)], op1=ALU.add)
                nc.vector.tensor_copy(out=xout[64:128, b, ts(t, 512)],
                                      in_=zn[:, 512:1024])
                nc.vector.scalar_tensor_tensor(
                    out=xout[64:128, b, ts(t, 512)],
                    in0=xout[64:128, b, ts(t, 512)],
                    scalar=bv_sb[64:128, bv_off + 1:bv_off + 2], op0=ALU.add,
                    in1=resid_ap[64:128, b, ts(t, 512)], op1=ALU.add)

            def seqnorm_b(xt, b):
                """Sequence-norm of [128, S] f32 (divide by unbiased var)."""
                stats = sb.tile([128, ST, 6], F32, tag="bnst", bufs=2,
                                name="stats")
                for g in range(ST):
                    nc.vector.bn_stats(out=stats[:, g, :],
                                       in_=xt[:, b, ts(g, 512)])
                mv = sb.tile([128, 2], F32, tag="bnmv", bufs=2, name="mv")
                nc.vector.bn_aggr(out=mv, in_=stats)
                r = sb.tile([128, 1], F32, tag="bnr", bufs=2, name="r")
                nc.vector.reciprocal(r, mv[:, 1:2])
                nc.vector.tensor_scalar(out=r, in0=r, scalar1=float(VARF),
                                        scalar2=None, op0=ALU.mult)
                mr = sb.tile([128, 1], F32, tag="bnmr", bufs=2, name="mr")
                nc.vector.scalar_tensor_tensor(
                    out=mr, in0=mv[:, 0:1], scalar=-1.0, op0=ALU.mult,
                    in1=r, op1=ALU.mult)
                nc.vector.scalar_tensor_tensor(
                    out=xt[:, b, :], in0=xt[:, b, :], scalar=r,
                    op0=ALU.mult, in1=mr.to_broadcast((128, S)),
                    op1=ALU.add)

            def ag_issue(xt, b, bb, fb):
                """Cast+store x[:, b, :] to the bounce buffer (SWDGE casting
                DMA on the gpsimd queue) and trigger the AllGather."""
                nc.gpsimd.dma_start(out=bb[:], in_=xt[:, b, :])
                nc.gpsimd.collective_compute(
                    "AllGather", ALU.bypass, replica_groups=RG,
                    ins=[bb[:]], outs=[fb[:]])

            # ================= sublayer 1: causal self-attention ===========
            for b in range(B):
                for t in range(ST):
                    qkv_tile(xTd.ap(), b, t, wq1, wk1, wv1, kT1, vO1, qt1,
                             qcol=0, kcol=1, bv_off=0)
            # prefetch enc tiles now (sync queue, streams during attn1)
            for b in range(B):
                for t in range(ST):
                    attn_tile(b, t, qt1, kT1, vO1, x1, resid, bv_off=0,
                              causal=True)
                if DBG:
                    nc.sync.dma_start(out=dbg_pre[:, b, :], in_=x1[:, b, :])
                seqnorm_b(x1, b)
                ag_issue(x1, b, x1b[b], x1f[b])
                # fill attn1(b)'s ACT-bound lag with enc K/V projections
                for t in range(ST):
                    qkv_tile(encd.ap(), b, t, None, wk2, wv2, kT2, vO2, None,
                             qcol=None, kcol=3, bv_off=HL)
            if DBG:
                for b in range(B):
                    nc.sync.dma_start(out=bview(dbg_x1)[:, b, :],
                                      in_=x1[:, b, :])
                nc.sync.dma_start(out=dbg_k1[:], in_=kT1)
                nc.sync.dma_start(out=dbg_q1[:], in_=qt1)
                nc.sync.dma_start(out=dbg_v1[:], in_=vO1)

            # ================= sublayer 2: cross-attention =================
            for b in range(B):
                x1f_v = x1f[b][:].rearrange("r p s -> p r s")
                for t in range(ST):
                    xs = sb.tile([128, KC, 512], F16, tag="xs", bufs=4,
                                 name="xs")
                    nc.sync.dma_start(out=xs, in_=x1f_v[:, :, ts(t, 512)])
                    proj128(xs, wq2, 2, qt2[:, b, t, :])
            for b in range(B):
                for t in range(ST):
                    attn_tile(b, t, qt2, kT2, vO2, x2, x1, bv_off=HL,
                              causal=False)
                seqnorm_b(x2, b)
                ag_issue(x2, b, x2b[b], x2f[b])
            if DBG:
                for b in range(B):
                    nc.sync.dma_start(out=bview(dbg_x2)[:, b, :],
                                      in_=x2[:, b, :])

            # ================= sublayer 3: FFN =============================
            for b in range(B):
                x2f_v = x2f[b][:].rearrange("r p s -> p r s")
                for t in range(ST):
                    xs = sb.tile([128, KC, 512], F16, tag="xs", bufs=4,
                                 name="xs")
                    nc.sync.dma_start(out=xs, in_=x2f_v[:, :, ts(t, 512)])
                    hT = sb.tile([128, FCL, 512], F16, tag="hT", bufs=2,
                                 name="hT")
                    for fc in range(FCL):
                        ps_h = pp.tile([128, 512], F32, tag="pp",
                                       name="ps_h")
                        for k in range(KC):
                            nc.tensor.matmul(ps_h, w1[:, k, ts(fc, 128)],
                                             xs[:, k, :],
                                             start=(k == 0),
                                             stop=(k == KC - 1))
                        nc.scalar.activation(
                            out=hT[:, fc, :], in_=ps_h, func=AF.Relu,
                            bias=b1_sb[:, fc:fc + 1], scale=1.0)
                    half = t // 2
                    rv = rsi[b][half][:]
                    for ec in range(KC):
                        ps_y = pp.tile([128, 512], F32, tag="pp",
                                       name="ps_y")
                        for fc in range(FCL):
                            nc.tensor.matmul(ps_y, w2[:, fc, ts(ec, 128)],
                                             hT[:, fc, :],
                                             start=(fc == 0),
                                             stop=(fc == FCL - 1))
                        ys = sb.tile([128, 512], F16, tag="ys", bufs=3,
                                     name="ys")
                        nc.scalar.activation(out=ys, in_=ps_y, func=AF.Copy)
                        nc.sync.dma_start(
                            out=rv[ec, :, ts(t % 2, 512)], in_=ys)
                    if t % 2 == 1:
                        nc.gpsimd.collective_compute(
                            "ReduceScatter", ALU.add, replica_groups=RG,
                            ins=[rsi[b][half][:]], outs=[rso[b][half][:]])

            # ======= y + b2 + x2 residual, seqnorm, write out (per b) ======
            x3 = sb.tile([128, B, S], F32, tag="xl", bufs=2, name="x3")
            for b in range(B):
                for half in range(2):
                    yh = sb.tile([128, SH], F16, tag="yh", bufs=2,
                                 name="yh")
                    nc.sync.dma_start(out=yh, in_=rso[b][half][:])
                    nc.vector.scalar_tensor_tensor(
                        out=x3[:, b, ts(half, SH)], in0=yh,
                        scalar=b2_sb[:, 0:1], op0=ALU.add,
                        in1=x2[:, b, ts(half, SH)], op1=ALU.add)
                seqnorm_b(x3, b)
                nc.sync.dma_start(out=bview(outT)[:, b, :], in_=x3[:, b, :])

    nc.compile()
    return nc


def _get_nc():
    global _CACHED_NC
    if _CACHED_NC is None:
        _CACHED_NC = _build()
    return _CACHED_NC


def _chunked(a):
    """[D, N] -> [128, D//128, N] with [p, c, n] = a[128c+p, n]."""
    d, n = a.shape
    return np.ascontiguousarray(
        a.reshape(d // 128, 128, n).transpose(1, 0, 2).astype(np.float16))


def _make_in_maps(decoder_input, encode_input,
                  Wq1, Wk1, Wv1, bq1, bk1, bv1,
                  Wq2, Wk2, Wv2, bq2, bk2, bv2,
                  W1, b1, W2, b2):
    xT = np.ascontiguousarray(
        np.transpose(np.asarray(decoder_input, np.float32), (0, 2, 1)))
    eT = np.transpose(np.asarray(encode_input, np.float32), (0, 2, 1))
    # [128, B, KC, S] fp16
    xTd_all = np.ascontiguousarray(
        xT.reshape(B, KC, 128, S).transpose(2, 0, 1, 3).astype(np.float16))
    encd_all = np.ascontiguousarray(
        eT.reshape(B, KC, 128, S).transpose(2, 0, 1, 3).astype(np.float16))
    in_maps = []
    for r in range(NCORES):
        hs = slice(DL * r, DL * (r + 1))
        fs = slice(FFL * r, FFL * (r + 1))
        resd = np.ascontiguousarray(
            xT[:, hs, :].transpose(1, 0, 2).astype(np.float16))
        bqk_arr = np.stack([bq1[hs], bk1[hs], bq2[hs], bk2[hs]],
                           axis=1).astype(np.float32)  # [128, 4]
        bv_arr = np.concatenate([
            bv1[hs].reshape(HL, DK).T, bv2[hs].reshape(HL, DK).T,
        ], axis=1).astype(np.float32)                  # [64, 4]
        bv_arr = np.concatenate([bv_arr, bv_arr], axis=0)  # [128, 4]
        in_maps.append({
            "xTd": xTd_all,
            "encd": encd_all,
            "resd": resd,
            "wq1d": _chunked(np.ascontiguousarray(Wq1[:, hs])),
            "wk1d": _chunked(np.ascontiguousarray(Wk1[:, hs])),
            "wv1d": _chunked(np.ascontiguousarray(Wv1[:, hs])),
            "wq2d": _chunked(np.ascontiguousarray(Wq2[:, hs])),
            "wk2d": _chunked(np.ascontiguousarray(Wk2[:, hs])),
            "wv2d": _chunked(np.ascontiguousarray(Wv2[:, hs])),
            "w1d": _chunked(np.ascontiguousarray(W1[:, fs])),
            "w2d": _chunked(np.ascontiguousarray(W2[fs, :])),
            "bqkd": bqk_arr,
            "bvd": bv_arr,
            "b1d": np.ascontiguousarray(
                b1[fs].reshape(FCL, 128).T.astype(np.float32)),
            "b2d": np.ascontiguousarray(
                b2[hs].reshape(128, 1).astype(np.float32)),
        })
    return in_maps


def kernel(**inputs):
    nc = _get_nc()
    in_maps = _make_in_maps(**{k: np.asarray(v) for k, v in inputs.items()})
    res = run_bass_kernel_spmd(nc, in_maps, core_ids=list(range(NCORES)),
                               trace=False)
    out = np.empty((B, S, D), np.float32)
    for r in range(NCORES):
        hs = slice(DL * r, DL * (r + 1))
        o = res.results[r]["outT"]                     # [B*DL, S]
        for b in range(B):
            out[b, :, hs] = o[b * DL:(b + 1) * DL].T
    return out


# revision 26
# speedup vs baseline: 1.0207x; 1.0207x over previous
"""Trainium2 Bass kernel for a 3-sublayer decoder block (nn_DecoderLayer).

Reference computation (B=2, S=2048, D=1024, H=16, DK=64, FF=4096, fp32):
  sa = causal_mha(x, x)          ; x1 = seqnorm(sa + x)
  ca = mha(x1, enc)              ; x2 = seqnorm(ca + x1)
  ffn = relu(x2 @ W1 + b1) @ W2 + b2 ; out = seqnorm(ffn + x2)
seqnorm normalizes over the SEQUENCE dim and divides by the unbiased VARIANCE
(reference quirk); attention has no output projection.

Sharding (8 cores, one replica group): every core processes BOTH batch
elements; heads split 2-per-core (a 128-wide channel slice of every sublayer
output per batch stays fully local, incl. seqnorm); FF hidden split
512-per-core (Megatron column/row). Collectives: AllGather(x1) and
AllGather(x2) per BATCH element (so AG(b0) hides under attention compute of
b1), ReduceScatter of FFN partials per (batch, seq-half) (so only the last
quarter-ish is tail-exposed).

v2 vs the fp32r baseline (1.44 ms):
 * fp16 operands end-to-end (host-cast): halves HBM+collective bytes, enables
   FWL fast weight loads (fp32 LDWEIGHTS was 423us of serialized PE time),
   and drops PE power (the fp32 version sat at K=4/8 = 1.2 GHz throttle).
 * per-batch collective pipelining (above) removes the ~310us of dead zones.
 * big streaming loads on the sync (HWDGE) queue, collective bounce stores as
   casting SWDGE DMAs on the gpsimd queue - so enc/x1f/x2f prefetch is never
   stuck behind an AllGather in the same in-order queue.
 * softmax denominator reciprocal via reciprocal_approx_fast (custom DVE op,
   ~5x faster than the iterative divide: that was 107us of DVE time).
 * PSUM: score tiles double-buffered (2x[128,1024]) so scores(c+1) overlaps
   exp(c) on ACT; PV accumulates both heads into one [128,1024] bank pair
   drained in a single copy.
All activations transposed on-chip ([d, s]); E^T = exp(K @ Q^T) computed
directly; softmax denominators from ones-columns appended to V.
"""

import os
import sys

import numpy as np

for _p in ("/opt/trn_rl_repo", "/root/.axon_site/_ro/trn_rl_repo"):
    if _p not in sys.path and os.path.isdir(_p):
        sys.path.append(_p)

import concourse.bass as bass
import concourse.mybir as mybir
import concourse.tile as tile
from concourse import bacc
from concourse.bass import ts
from concourse.bass_utils import run_bass_kernel_spmd

F32 = mybir.dt.float32
F16 = mybir.dt.float16
AF = mybir.ActivationFunctionType
ALU = mybir.AluOpType

B, S, D, H = 2, 2048, 1024, 16
DK = D // H            # 64
FF = 4 * D             # 4096
NCORES = 8
HL = H // NCORES       # 2 heads per core
DL = DK * HL           # 128 channels per core
FFL = FF // NCORES     # 512 ff dims per core
KC = D // 128          # 8 contraction chunks of the full model dim
FCL = FFL // 128       # 4 local ff chunks
ST = S // 512          # 4 sequence tiles of 512
SC = S // 128          # 16 sequence chunks of 128
SH = S // 2            # sequence half
SCALE = 1.0 / np.sqrt(DK)
VARF = (S - 1) / S     # unbiased-variance factor applied to 1/var_pop
VW = DK + 1            # 65: per-head stationary width in vO (v + ones col)

RG = [[0, 1, 2, 3, 4, 5, 6, 7]]

_CACHED_NC = None


def _build():
    nc = bacc.Bacc("TRN2", target_bir_lowering=False, debug=False,
                   num_devices=NCORES)

    # ---- per-core external inputs (fp16 except small fp32 biases) ----
    xTd = nc.dram_tensor("xTd", [128, B, KC, S], F16, kind="ExternalInput")
    encd = nc.dram_tensor("encd", [128, B, KC, S], F16, kind="ExternalInput")
    resd = nc.dram_tensor("resd", [128, B, S], F16, kind="ExternalInput")
    wq1d = nc.dram_tensor("wq1d", [128, KC, DL], F16, kind="ExternalInput")
    wk1d = nc.dram_tensor("wk1d", [128, KC, DL], F16, kind="ExternalInput")
    wv1d = nc.dram_tensor("wv1d", [128, KC, DL], F16, kind="ExternalInput")
    wq2d = nc.dram_tensor("wq2d", [128, KC, DL], F16, kind="ExternalInput")
    wk2d = nc.dram_tensor("wk2d", [128, KC, DL], F16, kind="ExternalInput")
    wv2d = nc.dram_tensor("wv2d", [128, KC, DL], F16, kind="ExternalInput")
    w1d = nc.dram_tensor("w1d", [128, KC, FFL], F16, kind="ExternalInput")
    w2d = nc.dram_tensor("w2d", [128, FCL, D], F16, kind="ExternalInput")
    bqkd = nc.dram_tensor("bqkd", [128, 4], F32, kind="ExternalInput")
    # bv values per (head, dk); rows 64:128 duplicate rows 0:64 so the h1
    # stt can use a partition-64-based scalar AP
    bvd = nc.dram_tensor("bvd", [128, 2 * HL], F32, kind="ExternalInput")
    b1d = nc.dram_tensor("b1d", [128, FCL], F32, kind="ExternalInput")
    b2d = nc.dram_tensor("b2d", [128, 1], F32, kind="ExternalInput")

    outT = nc.dram_tensor("outT", [B * DL, S], F32, kind="ExternalOutput")
    DBG = bool(os.environ.get("BASSDBG"))
    if DBG:
        dbg_x1 = nc.dram_tensor("dbg_x1", [B * DL, S], F32,
                                kind="ExternalOutput")
        dbg_x2 = nc.dram_tensor("dbg_x2", [B * DL, S], F32,
                                kind="ExternalOutput")
        dbg_k1 = nc.dram_tensor("dbg_k1", [128, B, S], F16,
                                kind="ExternalOutput")
        dbg_q1 = nc.dram_tensor("dbg_q1", [128, B, ST, 512], F16,
                                kind="ExternalOutput")
        dbg_v1 = nc.dram_tensor("dbg_v1", [128, B, SC, 2 * VW], F16,
                                kind="ExternalOutput")
        dbg_pre = nc.dram_tensor("dbg_pre", [128, B, S], F32,
                                 kind="ExternalOutput")

    def bview(t):   # [B*128, s] -> [p, b, s]
        return t[:].rearrange("(b p) s -> p b s", p=128)

    with tile.TileContext(nc) as tc:
        import contextlib
        ctx = contextlib.ExitStack()
        with ctx:
            sb = ctx.enter_context(tc.tile_pool(name="sb", bufs=1))
            dram = ctx.enter_context(tc.tile_pool(name="dr", bufs=1,
                                                  space="DRAM"))
            pp = ctx.enter_context(tc.tile_pool(name="pp", bufs=2,
                                                space="PSUM"))
            pe = ctx.enter_context(tc.tile_pool(name="pe", bufs=2,
                                                space="PSUM"))
            pz = ctx.enter_context(tc.tile_pool(name="pz", bufs=1,
                                                space="PSUM"))

            # ---- collective bounce buffers (per batch element) ----
            x1b = [dram.tile([DL, S], F16, tag=f"x1b{b}", name=f"x1b{b}")
                   for b in range(B)]
            x1f = [dram.tile([NCORES, DL, S], F16, tag=f"x1f{b}",
                             name=f"x1f{b}", addr_space="Shared")
                   for b in range(B)]
            x2b = [dram.tile([DL, S], F16, tag=f"x2b{b}", name=f"x2b{b}")
                   for b in range(B)]
            x2f = [dram.tile([NCORES, DL, S], F16, tag=f"x2f{b}",
                             name=f"x2f{b}", addr_space="Shared")
                   for b in range(B)]
            # RS per (b, seq-quarter): the last one is tail-exposed, so
            # keep the payloads small
            rsi = [[dram.tile([NCORES, DL, 512], F16, tag=f"rsi{b}{h}",
                              name=f"rsi{b}{h}") for h in range(ST)]
                   for b in range(B)]
            rso = [[dram.tile([DL, 512], F16, tag=f"rso{b}{h}",
                              name=f"rso{b}{h}")
                    for h in range(ST)] for b in range(B)]
            # dummy collective issued at t=0: the first collective on the
            # device carries the rank-sync barrier (~40-50us of launch skew);
            # firing it up front hides that under the warmup DMAs/compute
            dumb = dram.tile([128, 2], F16, tag="dumb", name="dumb")
            dumf = dram.tile([NCORES, 128, 2], F16, tag="dumf", name="dumf",
                             addr_space="Shared")
            nc.gpsimd.collective_compute(
                "AllGather", ALU.bypass, replica_groups=RG,
                ins=[dumb[:]], outs=[dumf[:]])

            # ---- small persistent tiles ----
            bqk_sb = sb.tile([128, 4], F32, tag="bias", bufs=1)
            nc.sync.dma_start(out=bqk_sb, in_=bqkd[:])
            bv_sb = sb.tile([128, 2 * HL], F32, tag="bias2", bufs=1)
            nc.sync.dma_start(out=bv_sb, in_=bvd[:])
            b1_sb = sb.tile([128, FCL], F32, tag="bias3", bufs=1)
            nc.sync.dma_start(out=b1_sb, in_=b1d[:])
            b2_sb = sb.tile([128, 1], F32, tag="bias4", bufs=1)
            nc.sync.dma_start(out=b2_sb, in_=b2d[:])

            # identity for PE transposes (fp16)
            id16 = sb.tile([128, 128], F16, tag="id16", bufs=1)
            nc.vector.memset(id16, 1.0)
            nc.gpsimd.affine_select(out=id16, in_=id16,
                                    compare_op=ALU.is_equal, fill=0.0,
                                    base=0, channel_multiplier=-1,
                                    pattern=[[1, 128]])

            # ---- weights (all loaded up front; ~3 MB fp16) ----
            def load_w(dram_t, name, cols, tag):
                w = sb.tile([128, KC, cols], F16, tag=tag, bufs=1, name=name)
                nc.sync.dma_start(out=w, in_=dram_t[:])
                return w

            wq1 = load_w(wq1d, "wq1", DL, "wqkv")
            wk1 = load_w(wk1d, "wk1", DL, "wqkv2")
            wv1 = load_w(wv1d, "wv1", DL, "wqkv3")
            wq2 = load_w(wq2d, "wq2", DL, "wqkv4")
            wk2 = load_w(wk2d, "wk2", DL, "wqkv5")
            wv2 = load_w(wv2d, "wv2", DL, "wqkv6")
            w1 = sb.tile([128, KC, FFL], F16, tag="w1", bufs=1, name="w1")
            nc.sync.dma_start(out=w1, in_=w1d[:])
            w2 = sb.tile([128, FCL, D], F16, tag="w2", bufs=1, name="w2")
            nc.sync.dma_start(out=w2, in_=w2d[:])

            resid = sb.tile([128, B, S], F16, tag="res", bufs=1, name="resid")
            nc.sync.dma_start(out=resid, in_=resd[:])

            # ---- persistent activation tiles ----
            kT1 = sb.tile([128, B, S], F16, tag="kT1", bufs=1, name="kT1")
            kT2 = sb.tile([128, B, S], F16, tag="kT2", bufs=1, name="kT2")
            # vO layout: [h0 ch(64) | ones | h1 ch(64) | ones] => stationary
            # for head h is the contiguous [128, 65] slice at 65*h.
            vO1 = sb.tile([128, B, SC, 2 * VW], F16, tag="vO1", bufs=1,
                          name="vO1")
            vO2 = sb.tile([128, B, SC, 2 * VW], F16, tag="vO2", bufs=1,
                          name="vO2")
            for vO in (vO1, vO2):
                nc.vector.memset(vO[:, :, :, DK:DK + 1], 1.0)
                nc.vector.memset(vO[:, :, :, DK + VW:DK + VW + 1], 1.0)
            qt1 = sb.tile([128, B, ST, 512], F16, tag="qt", bufs=2,
                          name="qt1")
            qt2 = sb.tile([128, B, ST, 512], F16, tag="qt", bufs=2,
                          name="qt2")
            x1 = sb.tile([128, B, S], F32, tag="xl", bufs=2, name="x1")
            x2 = sb.tile([128, B, S], F32, tag="xl", bufs=2, name="x2")

            def proj128(xs, w, bias_col, out_ap):
                """One [128, 512] projection: out = W.T @ x + bias (ACT
                drain psum->sbuf fp16 with per-channel bias)."""
                ps = pp.tile([128, 512], F32, tag="pp", name="ps")
                for k in range(KC):
                    nc.tensor.matmul(ps, w[:, k, :], xs[:, k, :],
                                     start=(k == 0), stop=(k == KC - 1))
                # drain on DVE: ACT is the bottleneck engine in attention
                nc.vector.tensor_scalar(
                    out=out_ap, in0=ps,
                    scalar1=bqk_sb[:, bias_col:bias_col + 1],
                    scalar2=None, op0=ALU.add)

            def load_xs(src_ap):
                xs = sb.tile([128, KC, 512], F16, tag="xs", bufs=4,
                             name="xs")
                nc.sync.dma_start(out=xs, in_=src_ap)
                return xs

            def qkv_tile(xs, b, t, wq, wk, wv, kT, vO, qt, qcol, kcol,
                         bv_off):
                """One (b, 512-seq-tile): q/k transposed projections, v
                projected then flipped back via PE transposes into vO (ones
                columns persist from the initial memset)."""
                if wq is not None:
                    proj128(xs, wq, qcol, qt[:, b, t, :])
                proj128(xs, wk, kcol, kT[:, b, ts(t, 512)])
                # vT [128(2h*dk), 512] (+bias) then transpose per 128-block
                ps = pp.tile([128, 512], F32, tag="pp", name="ps")
                for k in range(KC):
                    nc.tensor.matmul(ps, wv[:, k, :], xs[:, k, :],
                                     start=(k == 0), stop=(k == KC - 1))
                # no bias here: bv is added after softmax-normalize (rows of
                # P sum to 1, so PV(v)+bv == P(v+bv) normalized)
                vt = sb.tile([128, 512], F16, tag="vt", bufs=2, name="vt")
                nc.vector.tensor_copy(out=vt, in_=ps)
                tp = pp.tile([128, 512], F16, tag="pp", name="tp")
                for sc in range(4):
                    nc.tensor.transpose(tp[:, ts(sc, 128)],
                                        vt[:, ts(sc, 128)], id16)
                # one strided copy: [sc, h, dk] -> vO cols {0:64, 65:129}
                nc.vector.tensor_copy(
                    out=vO[:, b, 4 * t:4 * t + 4, :]
                        .rearrange("p c (h w) -> p c h w", h=2)[:, :, :, 0:DK],
                    in_=tp[:, :].rearrange("p (c h w) -> p c h w", c=4, h=2))

            def attn_tile(b, t, qt, kT, vO, xout, resid_ap, bv_off, causal):
                """One (b, sq-tile): per sk-chunk, E^T for both heads in one
                2-bank psum (disjoint PE row groups -> concurrent), exp on
                ACT (fp16 out), causal mask on gpsimd, PV for both heads into
                one 2-bank psum; then one drain + approx-reciprocal denom +
                normalize + bias + residual into xout."""
                nchunks = (4 * t + 4) if causal else SC
                zps = pz.tile([128, 1024], F32, tag="pz", name="zps")
                for c in range(nchunks):
                    eps = pe.tile([128, 1024], F32, tag="pe", name="eps")
                    for h in range(HL):
                        hb = h * 64
                        nc.tensor.matmul(eps[:, ts(h, 512)],
                                         kT[hb:hb + 64, b, ts(c, 128)],
                                         qt[hb:hb + 64, b, t, :],
                                         start=True, stop=True)
                    et = sb.tile([128, 1024], F16, tag="E", bufs=3,
                                 name="et")
                    nc.scalar.activation(out=et, in_=eps, func=AF.Exp,
                                         scale=float(SCALE))
                    if causal and c >= 4 * t:
                        j = c - 4 * t
                        for h in range(HL):
                            nc.gpsimd.affine_select(
                                out=et[:, ts(h, 512)],
                                in_=et[:, ts(h, 512)],
                                compare_op=ALU.is_ge,
                                fill=0.0, base=-(j * 128),
                                channel_multiplier=-1,
                                pattern=[[1, 512]])
                    for h in range(HL):
                        nc.tensor.matmul(
                            zps[0:VW, ts(h, 512)],
                            vO[:, b, c, VW * h:VW * h + VW],
                            et[:, ts(h, 512)],
                            start=(c == 0), stop=(c == nchunks - 1))
                # drain [65, 1024] once; row 64 = softmax denominators
                zsb = sb.tile([VW, 1024], F32, tag="zsb", bufs=2, name="zsb")
                nc.vector.tensor_copy(out=zsb, in_=zps[0:VW, :])
                dr = sb.tile([1, 1024], F32, tag="dr", bufs=2, name="dr")
                nc.vector.tensor_copy(out=dr, in_=zsb[DK:DK + 1, :])
                rb = sb.tile([64, 1024], F32, tag="rb", bufs=2, name="rb")
                nc.gpsimd.partition_broadcast(out_ap=rb, in_ap=dr)
                nc.vector.reciprocal_approx_fast(out=rb, in_=rb)
                zn = sb.tile([64, 1024], F32, tag="zn", bufs=2, name="zn")
                nc.vector.tensor_mul(zn, zsb[0:DK, :], rb)
                # h0 writes in place; h1 needs a partition shift (stt operands
                # must share a start partition; only copies may shift)
                nc.vector.scalar_tensor_tensor(
                    out=xout[0:64, b, ts(t, 512)],
                    in0=zn[:, 0:512],
                    scalar=bv_sb[0:64, bv_off:bv_off + 1], op0=ALU.add,
                    in1=resid_ap[0:64, b, ts(t, 512)], op1=ALU.add)
                nc.vector.tensor_copy(out=xout[64:128, b, ts(t, 512)],
                                      in_=zn[:, 512:1024])
                nc.vector.scalar_tensor_tensor(
                    out=xout[64:128, b, ts(t, 512)],
                    in0=xout[64:128, b, ts(t, 512)],
                    scalar=bv_sb[64:128, bv_off + 1:bv_off + 2], op0=ALU.add,
                    in1=resid_ap[64:128, b, ts(t, 512)], op1=ALU.add)

            def seqnorm_b(xt, b):
                """Sequence-norm of [128, S] f32 (divide by unbiased var)."""
                stats = sb.tile([128, ST, 6], F32, tag="bnst", bufs=2,
                                name="stats")
                for g in range(ST):
                    nc.vector.bn_stats(out=stats[:, g, :],
                                       in_=xt[:, b, ts(g, 512)])
                mv = sb.tile([128, 2], F32, tag="bnmv", bufs=2, name="mv")
                nc.vector.bn_aggr(out=mv, in_=stats)
                r = sb.tile([128, 1], F32, tag="bnr", bufs=2, name="r")
                nc.vector.reciprocal(r, mv[:, 1:2])
                nc.vector.tensor_scalar(out=r, in0=r, scalar1=float(VARF),
                                        scalar2=None, op0=ALU.mult)
                mr = sb.tile([128, 1], F32, tag="bnmr", bufs=2, name="mr")
                nc.vector.scalar_tensor_tensor(
                    out=mr, in0=mv[:, 0:1], scalar=-1.0, op0=ALU.mult,
                    in1=r, op1=ALU.mult)
                nc.vector.scalar_tensor_tensor(
                    out=xt[:, b, :], in0=xt[:, b, :], scalar=r,
                    op0=ALU.mult, in1=mr.to_broadcast((128, S)),
                    op1=ALU.add)

            def ag_issue(xt, b, bb, fb):
                """Cast+store x[:, b, :] to the bounce buffer (SWDGE casting
                DMA on the gpsimd queue) and trigger the AllGather."""
                nc.gpsimd.dma_start(out=bb[:], in_=xt[:, b, :])
                nc.gpsimd.collective_compute(
                    "AllGather", ALU.bypass, replica_groups=RG,
                    ins=[bb[:]], outs=[fb[:]])

            # ================= sublayer 1: causal self-attention ===========
            for b in range(B):
                for t in range(ST):
                    xs = load_xs(xTd.ap()[:, b, :, ts(t, 512)])
                    qkv_tile(xs, b, t, wq1, wk1, wv1, kT1, vO1, qt1,
                             qcol=0, kcol=1, bv_off=0)
            # prefetch enc tiles now (sync queue, streams during attn1)
            enc_xs = [[load_xs(encd.ap()[:, b, :, ts(t, 512)])
                       for t in range(ST)] for b in range(B)]
            q2_xs = [None] * B
            for b in range(B):
                for t in range(ST):
                    attn_tile(b, t, qt1, kT1, vO1, x1, resid, bv_off=0,
                              causal=True)
                if DBG:
                    nc.sync.dma_start(out=dbg_pre[:, b, :], in_=x1[:, b, :])
                seqnorm_b(x1, b)
                ag_issue(x1, b, x1b[b], x1f[b])
                # q2 loads for this b (gated on AG1(b); behind enc loads in
                # the sync queue, so enc prefetch is never blocked)
                x1f_v = x1f[b][:].rearrange("r p s -> p r s")
                q2_xs[b] = [load_xs(x1f_v[:, :, ts(t, 512)])
                            for t in range(ST)]
                # fill attn1(b)'s ACT-bound lag with enc K/V projections
                for t in range(ST):
                    qkv_tile(enc_xs[b][t], b, t, None, wk2, wv2, kT2, vO2,
                             None, qcol=None, kcol=3, bv_off=HL)
            if DBG:
                for b in range(B):
                    nc.sync.dma_start(out=bview(dbg_x1)[:, b, :],
                                      in_=x1[:, b, :])
                nc.sync.dma_start(out=dbg_k1[:], in_=kT1)
                nc.sync.dma_start(out=dbg_q1[:], in_=qt1)
                nc.sync.dma_start(out=dbg_v1[:], in_=vO1)

            # ================= sublayer 2: cross-attention =================
            for b in range(B):
                for t in range(ST):
                    proj128(q2_xs[b][t], wq2, 2, qt2[:, b, t, :])
            for b in range(B):
                for t in range(ST):
                    attn_tile(b, t, qt2, kT2, vO2, x2, x1, bv_off=HL,
                              causal=False)
                seqnorm_b(x2, b)
                ag_issue(x2, b, x2b[b], x2f[b])
            if DBG:
                for b in range(B):
                    nc.sync.dma_start(out=bview(dbg_x2)[:, b, :],
                                      in_=x2[:, b, :])

            # ================= sublayer 3: FFN =============================
            for b in range(B):
                x2f_v = x2f[b][:].rearrange("r p s -> p r s")
                for t in range(ST):
                    xs = load_xs(x2f_v[:, :, ts(t, 512)])
                    hT = sb.tile([128, FCL, 512], F16, tag="hT", bufs=2,
                                 name="hT")
                    for fc in range(FCL):
                        ps_h = pp.tile([128, 512], F32, tag="pp",
                                       name="ps_h")
                        for k in range(KC):
                            nc.tensor.matmul(ps_h, w1[:, k, ts(fc, 128)],
                                             xs[:, k, :],
                                             start=(k == 0),
                                             stop=(k == KC - 1))
                        # relu(x + b1) on DVE: (in + b1) max 0
                        nc.vector.tensor_scalar(
                            out=hT[:, fc, :], in0=ps_h,
                            scalar1=b1_sb[:, fc:fc + 1], scalar2=0.0,
                            op0=ALU.add, op1=ALU.max)
                    rv = rsi[b][t][:]
                    for ec in range(KC):
                        ps_y = pp.tile([128, 512], F32, tag="pp",
                                       name="ps_y")
                        for fc in range(FCL):
                            nc.tensor.matmul(ps_y, w2[:, fc, ts(ec, 128)],
                                             hT[:, fc, :],
                                             start=(fc == 0),
                                             stop=(fc == FCL - 1))
                        ys = sb.tile([128, 512], F16, tag="ys", bufs=3,
                                     name="ys")
                        nc.vector.tensor_copy(out=ys, in_=ps_y)
                        nc.sync.dma_start(out=rv[ec, :, :], in_=ys)
                    nc.gpsimd.collective_compute(
                        "ReduceScatter", ALU.add, replica_groups=RG,
                        ins=[rsi[b][t][:]], outs=[rso[b][t][:]])

            # ======= y + b2 + x2 residual, seqnorm, write out (per b) ======
            # per-quarter assembly + stats so only the last RS is exposed
            x3 = sb.tile([128, B, S], F32, tag="xl", bufs=2, name="x3")
            for b in range(B):
                stats = sb.tile([128, ST, 6], F32, tag="bnst", bufs=2,
                                name="stats")
                for t in range(ST):
                    yh = sb.tile([128, 512], F16, tag="yh", bufs=2,
                                 name="yh")
                    nc.sync.dma_start(out=yh, in_=rso[b][t][:])
                    nc.vector.scalar_tensor_tensor(
                        out=x3[:, b, ts(t, 512)], in0=yh,
                        scalar=b2_sb[:, 0:1], op0=ALU.add,
                        in1=x2[:, b, ts(t, 512)], op1=ALU.add)
                    nc.vector.bn_stats(out=stats[:, t, :],
                                       in_=x3[:, b, ts(t, 512)])
                mv = sb.tile([128, 2], F32, tag="bnmv", bufs=2, name="mv")
                nc.vector.bn_aggr(out=mv, in_=stats)
                r = sb.tile([128, 1], F32, tag="bnr", bufs=2, name="r")
                nc.vector.reciprocal(r, mv[:, 1:2])
                nc.vector.tensor_scalar(out=r, in0=r, scalar1=float(VARF),
                                        scalar2=None, op0=ALU.mult)
                mr = sb.tile([128, 1], F32, tag="bnmr", bufs=2, name="mr")
                nc.vector.scalar_tensor_tensor(
                    out=mr, in0=mv[:, 0:1], scalar=-1.0, op0=ALU.mult,
                    in1=r, op1=ALU.mult)
                nc.vector.scalar_tensor_tensor(
                    out=x3[:, b, :], in0=x3[:, b, :], scalar=r,
                    op0=ALU.mult, in1=mr.to_broadcast((128, S)),
                    op1=ALU.add)
                nc.sync.dma_start(out=bview(outT)[:, b, :], in_=x3[:, b, :])

    nc.compile()
    return nc


def _get_nc():
    global _CACHED_NC
    if _CACHED_NC is None:
        _CACHED_NC = _build()
    return _CACHED_NC


def _chunked(a):
    """[D, N] -> [128, D//128, N] with [p, c, n] = a[128c+p, n]."""
    d, n = a.shape
    return np.ascontiguousarray(
        a.reshape(d // 128, 128, n).transpose(1, 0, 2).astype(np.float16))


def _make_in_maps(decoder_input, encode_input,
                  Wq1, Wk1, Wv1, bq1, bk1, bv1,
                  Wq2, Wk2, Wv2, bq2, bk2, bv2,
                  W1, b1, W2, b2):
    xT = np.ascontiguousarray(
        np.transpose(np.asarray(decoder_input, np.float32), (0, 2, 1)))
    eT = np.transpose(np.asarray(encode_input, np.float32), (0, 2, 1))
    # [128, B, KC, S] fp16
    xTd_all = np.ascontiguousarray(
        xT.reshape(B, KC, 128, S).transpose(2, 0, 1, 3).astype(np.float16))
    encd_all = np.ascontiguousarray(
        eT.reshape(B, KC, 128, S).transpose(2, 0, 1, 3).astype(np.float16))
    in_maps = []
    for r in range(NCORES):
        hs = slice(DL * r, DL * (r + 1))
        fs = slice(FFL * r, FFL * (r + 1))
        resd = np.ascontiguousarray(
            xT[:, hs, :].transpose(1, 0, 2).astype(np.float16))
        bqk_arr = np.stack([bq1[hs], bk1[hs], bq2[hs], bk2[hs]],
                           axis=1).astype(np.float32)  # [128, 4]
        bv_arr = np.concatenate([
            bv1[hs].reshape(HL, DK).T, bv2[hs].reshape(HL, DK).T,
        ], axis=1).astype(np.float32)                  # [64, 4]
        bv_arr = np.concatenate([bv_arr, bv_arr], axis=0)  # [128, 4]
        in_maps.append({
            "xTd": xTd_all,
            "encd": encd_all,
            "resd": resd,
            "wq1d": _chunked(np.ascontiguousarray(Wq1[:, hs])),
            "wk1d": _chunked(np.ascontiguousarray(Wk1[:, hs])),
            "wv1d": _chunked(np.ascontiguousarray(Wv1[:, hs])),
            "wq2d": _chunked(np.ascontiguousarray(Wq2[:, hs])),
            "wk2d": _chunked(np.ascontiguousarray(Wk2[:, hs])),
            "wv2d": _chunked(np.ascontiguousarray(Wv2[:, hs])),
            "w1d": _chunked(np.ascontiguousarray(W1[:, fs])),
            "w2d": _chunked(np.ascontiguousarray(W2[fs, :])),
            "bqkd": bqk_arr,
            "bvd": bv_arr,
            "b1d": np.ascontiguousarray(
                b1[fs].reshape(FCL, 128).T.astype(np.float32)),
            "b2d": np.ascontiguousarray(
                b2[hs].reshape(128, 1).astype(np.float32)),
        })
    return in_maps


def kernel(**inputs):
    nc = _get_nc()
    in_maps = _make_in_maps(**{k: np.asarray(v) for k, v in inputs.items()})
    res = run_bass_kernel_spmd(nc, in_maps, core_ids=list(range(NCORES)),
                               trace=False)
    out = np.empty((B, S, D), np.float32)
    for r in range(NCORES):
        hs = slice(DL * r, DL * (r + 1))
        o = res.results[r]["outT"]                     # [B*DL, S]
        for b in range(B):
            out[b, :, hs] = o[b * DL:(b + 1) * DL].T
    return out


# revision 34
# speedup vs baseline: 1.0506x; 1.0293x over previous
"""Trainium2 Bass kernel for a 3-sublayer decoder block (nn_DecoderLayer).

Reference computation (B=2, S=2048, D=1024, H=16, DK=64, FF=4096, fp32):
  sa = causal_mha(x, x)          ; x1 = seqnorm(sa + x)
  ca = mha(x1, enc)              ; x2 = seqnorm(ca + x1)
  ffn = relu(x2 @ W1 + b1) @ W2 + b2 ; out = seqnorm(ffn + x2)
seqnorm normalizes over the SEQUENCE dim and divides by the unbiased VARIANCE
(reference quirk); attention has no output projection.

Sharding (8 cores, one replica group): every core processes BOTH batch
elements; heads split 2-per-core (a 128-wide channel slice of every sublayer
output per batch stays fully local, incl. seqnorm); FF hidden split
512-per-core (Megatron column/row). Collectives: AllGather(x1) and
AllGather(x2) per BATCH element (so AG(b0) hides under attention compute of
b1), ReduceScatter of FFN partials per (batch, seq-half) (so only the last
quarter-ish is tail-exposed).

v2 vs the fp32r baseline (1.44 ms):
 * fp16 operands end-to-end (host-cast): halves HBM+collective bytes, enables
   FWL fast weight loads (fp32 LDWEIGHTS was 423us of serialized PE time),
   and drops PE power (the fp32 version sat at K=4/8 = 1.2 GHz throttle).
 * per-batch collective pipelining (above) removes the ~310us of dead zones.
 * big streaming loads on the sync (HWDGE) queue, collective bounce stores as
   casting SWDGE DMAs on the gpsimd queue - so enc/x1f/x2f prefetch is never
   stuck behind an AllGather in the same in-order queue.
 * softmax denominator reciprocal via reciprocal_approx_fast (custom DVE op,
   ~5x faster than the iterative divide: that was 107us of DVE time).
 * PSUM: score tiles double-buffered (2x[128,1024]) so scores(c+1) overlaps
   exp(c) on ACT; PV accumulates both heads into one [128,1024] bank pair
   drained in a single copy.
All activations transposed on-chip ([d, s]); E^T = exp(K @ Q^T) computed
directly; softmax denominators from ones-columns appended to V.
"""

import os
import sys

import numpy as np

for _p in ("/opt/trn_rl_repo", "/root/.axon_site/_ro/trn_rl_repo"):
    if _p not in sys.path and os.path.isdir(_p):
        sys.path.append(_p)

import concourse.bass as bass
import concourse.mybir as mybir
import concourse.tile as tile
from concourse import bacc
from concourse.bass import ts
from concourse.bass_utils import run_bass_kernel_spmd

F32 = mybir.dt.float32
F16 = mybir.dt.float16
AF = mybir.ActivationFunctionType
ALU = mybir.AluOpType

B, S, D, H = 2, 2048, 1024, 16
DK = D // H            # 64
FF = 4 * D             # 4096
NCORES = 8
HL = H // NCORES       # 2 heads per core
DL = DK * HL           # 128 channels per core
FFL = FF // NCORES     # 512 ff dims per core
KC = D // 128          # 8 contraction chunks of the full model dim
FCL = FFL // 128       # 4 local ff chunks
ST = S // 512          # 4 sequence tiles of 512
SC = S // 128          # 16 sequence chunks of 128
SH = S // 2            # sequence half
SCALE = 1.0 / np.sqrt(DK)
VARF = (S - 1) / S     # unbiased-variance factor applied to 1/var_pop
VW = DK + 1            # 65: per-head stationary width in vO (v + ones col)

RG = [[0, 1, 2, 3, 4, 5, 6, 7]]

_CACHED_NC = None


def _build():
    nc = bacc.Bacc("TRN2", target_bir_lowering=False, debug=False,
                   num_devices=NCORES)

    # ---- per-core external inputs (fp16 except small fp32 biases) ----
    xTd = nc.dram_tensor("xTd", [128, B, KC, S], F16, kind="ExternalInput")
    encd = nc.dram_tensor("encd", [128, B, KC, S], F16, kind="ExternalInput")
    resd = nc.dram_tensor("resd", [128, B, S], F16, kind="ExternalInput")
    wq1d = nc.dram_tensor("wq1d", [128, KC, DL], F16, kind="ExternalInput")
    wk1d = nc.dram_tensor("wk1d", [128, KC, DL], F16, kind="ExternalInput")
    wv1d = nc.dram_tensor("wv1d", [128, KC, DL], F16, kind="ExternalInput")
    wq2d = nc.dram_tensor("wq2d", [128, KC, DL], F16, kind="ExternalInput")
    wk2d = nc.dram_tensor("wk2d", [128, KC, DL], F16, kind="ExternalInput")
    wv2d = nc.dram_tensor("wv2d", [128, KC, DL], F16, kind="ExternalInput")
    w1d = nc.dram_tensor("w1d", [128, KC, FFL], F16, kind="ExternalInput")
    w2d = nc.dram_tensor("w2d", [128, FCL, D], F16, kind="ExternalInput")
    bqkd = nc.dram_tensor("bqkd", [128, 4], F32, kind="ExternalInput")
    # bv values per (head, dk); rows 64:128 duplicate rows 0:64 so the h1
    # stt can use a partition-64-based scalar AP
    bvd = nc.dram_tensor("bvd", [128, 2 * HL], F32, kind="ExternalInput")
    b1d = nc.dram_tensor("b1d", [128, FCL], F32, kind="ExternalInput")
    b2d = nc.dram_tensor("b2d", [128, 1], F32, kind="ExternalInput")

    outT = nc.dram_tensor("outT", [B * DL, S], F32, kind="ExternalOutput")
    DBG = bool(os.environ.get("BASSDBG"))
    if DBG:
        dbg_x1 = nc.dram_tensor("dbg_x1", [B * DL, S], F32,
                                kind="ExternalOutput")
        dbg_x2 = nc.dram_tensor("dbg_x2", [B * DL, S], F32,
                                kind="ExternalOutput")
        dbg_k1 = nc.dram_tensor("dbg_k1", [128, B, S], F16,
                                kind="ExternalOutput")
        dbg_q1 = nc.dram_tensor("dbg_q1", [128, B, ST, 512], F16,
                                kind="ExternalOutput")
        dbg_v1 = nc.dram_tensor("dbg_v1", [128, B, SC, 2 * VW], F16,
                                kind="ExternalOutput")
        dbg_pre = nc.dram_tensor("dbg_pre", [128, B, S], F32,
                                 kind="ExternalOutput")

    def bview(t):   # [B*128, s] -> [p, b, s]
        return t[:].rearrange("(b p) s -> p b s", p=128)

    with tile.TileContext(nc) as tc:
        import contextlib
        ctx = contextlib.ExitStack()
        with ctx:
            sb = ctx.enter_context(tc.tile_pool(name="sb", bufs=1))
            dram = ctx.enter_context(tc.tile_pool(name="dr", bufs=1,
                                                  space="DRAM"))
            pp = ctx.enter_context(tc.tile_pool(name="pp", bufs=2,
                                                space="PSUM"))
            pe = ctx.enter_context(tc.tile_pool(name="pe", bufs=2,
                                                space="PSUM"))
            pz = ctx.enter_context(tc.tile_pool(name="pz", bufs=1,
                                                space="PSUM"))

            # ---- collective bounce buffers (per batch element) ----
            x1b = [dram.tile([DL, S], F16, tag=f"x1b{b}", name=f"x1b{b}")
                   for b in range(B)]
            x1f = [dram.tile([NCORES, DL, S], F16, tag=f"x1f{b}",
                             name=f"x1f{b}", addr_space="Shared")
                   for b in range(B)]
            x2b = [dram.tile([DL, S], F16, tag=f"x2b{b}", name=f"x2b{b}")
                   for b in range(B)]
            x2f = [dram.tile([NCORES, DL, S], F16, tag=f"x2f{b}",
                             name=f"x2f{b}", addr_space="Shared")
                   for b in range(B)]
            # RS per (b, seq-half): few enough that the serial cc stream
            # doesn't congest, small enough that only ~one is tail-exposed
            rsi = [[dram.tile([NCORES, DL, SH], F16, tag=f"rsi{b}{h}",
                              name=f"rsi{b}{h}") for h in range(2)]
                   for b in range(B)]
            rso = [[dram.tile([DL, SH], F16, tag=f"rso{b}{h}",
                              name=f"rso{b}{h}")
                    for h in range(2)] for b in range(B)]
            # dummy collective issued at t=0: the first collective on the
            # device carries the rank-sync barrier (~40-50us of launch skew);
            # firing it up front hides that under the warmup DMAs/compute
            dumb = dram.tile([128, 2], F16, tag="dumb", name="dumb")
            dumf = dram.tile([NCORES, 128, 2], F16, tag="dumf", name="dumf",
                             addr_space="Shared")
            nc.gpsimd.collective_compute(
                "AllGather", ALU.bypass, replica_groups=RG,
                ins=[dumb[:]], outs=[dumf[:]])

            # ---- small persistent tiles ----
            bqk_sb = sb.tile([128, 4], F32, tag="bias", bufs=1)
            nc.sync.dma_start(out=bqk_sb, in_=bqkd[:])
            bv_sb = sb.tile([128, 2 * HL], F32, tag="bias2", bufs=1)
            nc.sync.dma_start(out=bv_sb, in_=bvd[:])
            b1_sb = sb.tile([128, FCL], F32, tag="bias3", bufs=1)
            nc.sync.dma_start(out=b1_sb, in_=b1d[:])
            b2_sb = sb.tile([128, 1], F32, tag="bias4", bufs=1)
            nc.sync.dma_start(out=b2_sb, in_=b2d[:])

            # identity for PE transposes (fp16)
            id16 = sb.tile([128, 128], F16, tag="id16", bufs=1)
            nc.vector.memset(id16, 1.0)
            nc.gpsimd.affine_select(out=id16, in_=id16,
                                    compare_op=ALU.is_equal, fill=0.0,
                                    base=0, channel_multiplier=-1,
                                    pattern=[[1, 128]])

            # ---- weights (all loaded up front; ~3 MB fp16) ----
            def load_w(dram_t, name, cols, tag):
                w = sb.tile([128, KC, cols], F16, tag=tag, bufs=1, name=name)
                nc.sync.dma_start(out=w, in_=dram_t[:])
                return w

            # only sublayer-1 weights now; the rest are loaded after the
            # qkv1 xs loads are queued so the first matmul starts early
            wq1 = load_w(wq1d, "wq1", DL, "wqkv")
            wk1 = load_w(wk1d, "wk1", DL, "wqkv2")
            wv1 = load_w(wv1d, "wv1", DL, "wqkv3")

            # ---- persistent activation tiles ----
            kT1 = sb.tile([128, B, S], F16, tag="kT1", bufs=1, name="kT1")
            kT2 = sb.tile([128, B, S], F16, tag="kT2", bufs=1, name="kT2")
            # vO layout: [h0 ch(64) | ones | h1 ch(64) | ones] => stationary
            # for head h is the contiguous [128, 65] slice at 65*h.
            vO1 = sb.tile([128, B, SC, 2 * VW], F16, tag="vO1", bufs=1,
                          name="vO1")
            vO2 = sb.tile([128, B, SC, 2 * VW], F16, tag="vO2", bufs=1,
                          name="vO2")
            for vO in (vO1, vO2):
                nc.vector.memset(vO[:, :, :, DK:DK + 1], 1.0)
                nc.vector.memset(vO[:, :, :, DK + VW:DK + VW + 1], 1.0)
            qt1 = sb.tile([128, B, ST, 512], F16, tag="qt", bufs=2,
                          name="qt1")
            qt2 = sb.tile([128, B, ST, 512], F16, tag="qt", bufs=2,
                          name="qt2")
            x1 = sb.tile([128, B, S], F32, tag="xl", bufs=2, name="x1")
            x2 = sb.tile([128, B, S], F32, tag="xl", bufs=2, name="x2")

            def proj128(xs, w, bias_col, out_ap):
                """One [128, 512] projection: out = W.T @ x + bias (ACT
                drain psum->sbuf fp16 with per-channel bias)."""
                ps = pp.tile([128, 512], F32, tag="pp", name="ps")
                for k in range(KC):
                    nc.tensor.matmul(ps, w[:, k, :], xs[:, k, :],
                                     start=(k == 0), stop=(k == KC - 1))
                # drain on DVE: ACT is the bottleneck engine in attention
                nc.vector.tensor_scalar(
                    out=out_ap, in0=ps,
                    scalar1=bqk_sb[:, bias_col:bias_col + 1],
                    scalar2=None, op0=ALU.add)

            def load_xs(src_ap):
                xs = sb.tile([128, KC, 512], F16, tag="xs", bufs=4,
                             name="xs")
                nc.sync.dma_start(out=xs, in_=src_ap)
                return xs

            def qkv_tile(xs, b, t, wq, wk, wv, kT, vO, qt, qcol, kcol,
                         bv_off):
                """One (b, 512-seq-tile): q/k transposed projections, v
                projected then flipped back via PE transposes into vO (ones
                columns persist from the initial memset)."""
                if wq is not None:
                    proj128(xs, wq, qcol, qt[:, b, t, :])
                proj128(xs, wk, kcol, kT[:, b, ts(t, 512)])
                # vT [128(2h*dk), 512] (+bias) then transpose per 128-block
                ps = pp.tile([128, 512], F32, tag="pp", name="ps")
                for k in range(KC):
                    nc.tensor.matmul(ps, wv[:, k, :], xs[:, k, :],
                                     start=(k == 0), stop=(k == KC - 1))
                # no bias here: bv is added after softmax-normalize (rows of
                # P sum to 1, so PV(v)+bv == P(v+bv) normalized)
                vt = sb.tile([128, 512], F16, tag="vt", bufs=2, name="vt")
                nc.vector.tensor_copy(out=vt, in_=ps)
                tp = pp.tile([128, 512], F16, tag="pp", name="tp")
                for sc in range(4):
                    nc.tensor.transpose(tp[:, ts(sc, 128)],
                                        vt[:, ts(sc, 128)], id16)
                # one strided copy: [sc, h, dk] -> vO cols {0:64, 65:129}
                nc.vector.tensor_copy(
                    out=vO[:, b, 4 * t:4 * t + 4, :]
                        .rearrange("p c (h w) -> p c h w", h=2)[:, :, :, 0:DK],
                    in_=tp[:, :].rearrange("p (c h w) -> p c h w", c=4, h=2))

            def attn_tile(b, t, qt, kT, vO, xout, resid_ap, bv_off, causal):
                """One (b, sq-tile): per sk-chunk, E^T for both heads in one
                2-bank psum (disjoint PE row groups -> concurrent), exp on
                ACT (fp16 out), causal mask on gpsimd, PV for both heads into
                one 2-bank psum; then one drain + approx-reciprocal denom +
                normalize + bias + residual into xout."""
                nchunks = (4 * t + 4) if causal else SC
                zps = pz.tile([128, 1024], F32, tag="pz", name="zps")
                for c in range(nchunks):
                    eps = pe.tile([128, 1024], F32, tag="pe", name="eps")
                    for h in range(HL):
                        hb = h * 64
                        nc.tensor.matmul(eps[:, ts(h, 512)],
                                         kT[hb:hb + 64, b, ts(c, 128)],
                                         qt[hb:hb + 64, b, t, :],
                                         start=True, stop=True)
                    et = sb.tile([128, 1024], F16, tag="E", bufs=3,
                                 name="et")
                    nc.scalar.activation(out=et, in_=eps, func=AF.Exp,
                                         scale=float(SCALE))
                    if causal and c >= 4 * t:
                        j = c - 4 * t
                        for h in range(HL):
                            nc.gpsimd.affine_select(
                                out=et[:, ts(h, 512)],
                                in_=et[:, ts(h, 512)],
                                compare_op=ALU.is_ge,
                                fill=0.0, base=-(j * 128),
                                channel_multiplier=-1,
                                pattern=[[1, 512]])
                    for h in range(HL):
                        nc.tensor.matmul(
                            zps[0:VW, ts(h, 512)],
                            vO[:, b, c, VW * h:VW * h + VW],
                            et[:, ts(h, 512)],
                            start=(c == 0), stop=(c == nchunks - 1))
                # drain [65, 1024] once; row 64 = softmax denominators
                zsb = sb.tile([VW, 1024], F32, tag="zsb", bufs=2, name="zsb")
                nc.vector.tensor_copy(out=zsb, in_=zps[0:VW, :])
                dr = sb.tile([1, 1024], F32, tag="dr", bufs=2, name="dr")
                nc.vector.tensor_copy(out=dr, in_=zsb[DK:DK + 1, :])
                rb = sb.tile([64, 1024], F32, tag="rb", bufs=2, name="rb")
                nc.gpsimd.partition_broadcast(out_ap=rb, in_ap=dr)
                nc.vector.reciprocal_approx_fast(out=rb, in_=rb)
                zn = sb.tile([64, 1024], F32, tag="zn", bufs=2, name="zn")
                nc.vector.tensor_mul(zn, zsb[0:DK, :], rb)
                # h0 writes in place; h1 needs a partition shift (stt operands
                # must share a start partition; only copies may shift)
                nc.vector.scalar_tensor_tensor(
                    out=xout[0:64, b, ts(t, 512)],
                    in0=zn[:, 0:512],
                    scalar=bv_sb[0:64, bv_off:bv_off + 1], op0=ALU.add,
                    in1=resid_ap[0:64, b, ts(t, 512)], op1=ALU.add)
                nc.vector.tensor_copy(out=xout[64:128, b, ts(t, 512)],
                                      in_=zn[:, 512:1024])
                nc.vector.scalar_tensor_tensor(
                    out=xout[64:128, b, ts(t, 512)],
                    in0=xout[64:128, b, ts(t, 512)],
                    scalar=bv_sb[64:128, bv_off + 1:bv_off + 2], op0=ALU.add,
                    in1=resid_ap[64:128, b, ts(t, 512)], op1=ALU.add)

            def seqnorm_b(xt, b):
                """Sequence-norm of [128, S] f32 (divide by unbiased var)."""
                stats = sb.tile([128, ST, 6], F32, tag="bnst", bufs=2,
                                name="stats")
                for g in range(ST):
                    nc.vector.bn_stats(out=stats[:, g, :],
                                       in_=xt[:, b, ts(g, 512)])
                mv = sb.tile([128, 2], F32, tag="bnmv", bufs=2, name="mv")
                nc.vector.bn_aggr(out=mv, in_=stats)
                r = sb.tile([128, 1], F32, tag="bnr", bufs=2, name="r")
                nc.vector.reciprocal(r, mv[:, 1:2])
                nc.vector.tensor_scalar(out=r, in0=r, scalar1=float(VARF),
                                        scalar2=None, op0=ALU.mult)
                mr = sb.tile([128, 1], F32, tag="bnmr", bufs=2, name="mr")
                nc.vector.scalar_tensor_tensor(
                    out=mr, in0=mv[:, 0:1], scalar=-1.0, op0=ALU.mult,
                    in1=r, op1=ALU.mult)
                nc.vector.scalar_tensor_tensor(
                    out=xt[:, b, :], in0=xt[:, b, :], scalar=r,
                    op0=ALU.mult, in1=mr.to_broadcast((128, S)),
                    op1=ALU.add)

            def ag_issue(xt, b, bb, fb):
                """Cast+store x[:, b, :] to the bounce buffer (SWDGE casting
                DMA on the gpsimd queue) and trigger the AllGather."""
                nc.gpsimd.dma_start(out=bb[:], in_=xt[:, b, :])
                nc.gpsimd.collective_compute(
                    "AllGather", ALU.bypass, replica_groups=RG,
                    ins=[bb[:]], outs=[fb[:]])

            # ================= sublayer 1: causal self-attention ===========
            for b in range(B):
                for t in range(ST):
                    xs = load_xs(xTd.ap()[:, b, :, ts(t, 512)])
                    qkv_tile(xs, b, t, wq1, wk1, wv1, kT1, vO1, qt1,
                             qcol=0, kcol=1, bv_off=0)
            # deferred loads: residual + later-phase weights queue behind
            # the qkv1 xs loads
            resid = sb.tile([128, B, S], F16, tag="res", bufs=1,
                            name="resid")
            nc.sync.dma_start(out=resid, in_=resd[:])
            wq2 = load_w(wq2d, "wq2", DL, "wqkv4")
            wk2 = load_w(wk2d, "wk2", DL, "wqkv5")
            wv2 = load_w(wv2d, "wv2", DL, "wqkv6")
            w1 = sb.tile([128, KC, FFL], F16, tag="w1", bufs=1, name="w1")
            nc.sync.dma_start(out=w1, in_=w1d[:])
            w2 = sb.tile([128, FCL, D], F16, tag="w2", bufs=1, name="w2")
            nc.sync.dma_start(out=w2, in_=w2d[:])
            # prefetch enc tiles now (sync queue, streams during attn1)
            enc_xs = [[load_xs(encd.ap()[:, b, :, ts(t, 512)])
                       for t in range(ST)] for b in range(B)]
            q2_xs = [None] * B
            for b in range(B):
                for t in range(ST):
                    attn_tile(b, t, qt1, kT1, vO1, x1, resid, bv_off=0,
                              causal=True)
                if DBG:
                    nc.sync.dma_start(out=dbg_pre[:, b, :], in_=x1[:, b, :])
                seqnorm_b(x1, b)
                ag_issue(x1, b, x1b[b], x1f[b])
                # q2 loads for this b (gated on AG1(b); behind enc loads in
                # the sync queue, so enc prefetch is never blocked)
                x1f_v = x1f[b][:].rearrange("r p s -> p r s")
                q2_xs[b] = [load_xs(x1f_v[:, :, ts(t, 512)])
                            for t in range(ST)]
                # fill attn1(b)'s ACT-bound lag with enc K/V projections
                for t in range(ST):
                    qkv_tile(enc_xs[b][t], b, t, None, wk2, wv2, kT2, vO2,
                             None, qcol=None, kcol=3, bv_off=HL)
            if DBG:
                for b in range(B):
                    nc.sync.dma_start(out=bview(dbg_x1)[:, b, :],
                                      in_=x1[:, b, :])
                nc.sync.dma_start(out=dbg_k1[:], in_=kT1)
                nc.sync.dma_start(out=dbg_q1[:], in_=qt1)
                nc.sync.dma_start(out=dbg_v1[:], in_=vO1)

            # ================= sublayer 2: cross-attention =================
            # q2 projection interleaved per tile so ACT's exp stream starts
            # after a single projection instead of all eight
            for b in range(B):
                for t in range(ST):
                    proj128(q2_xs[b][t], wq2, 2, qt2[:, b, t, :])
                    attn_tile(b, t, qt2, kT2, vO2, x2, x1, bv_off=HL,
                              causal=False)
                seqnorm_b(x2, b)
                ag_issue(x2, b, x2b[b], x2f[b])
            if DBG:
                for b in range(B):
                    nc.sync.dma_start(out=bview(dbg_x2)[:, b, :],
                                      in_=x2[:, b, :])

            # ================= sublayer 3: FFN =============================
            for b in range(B):
                x2f_v = x2f[b][:].rearrange("r p s -> p r s")
                for t in range(ST):
                    xs = load_xs(x2f_v[:, :, ts(t, 512)])
                    hT = sb.tile([128, FCL, 512], F16, tag="hT", bufs=2,
                                 name="hT")
                    for fc in range(FCL):
                        ps_h = pp.tile([128, 512], F32, tag="pp",
                                       name="ps_h")
                        for k in range(KC):
                            nc.tensor.matmul(ps_h, w1[:, k, ts(fc, 128)],
                                             xs[:, k, :],
                                             start=(k == 0),
                                             stop=(k == KC - 1))
                        # relu(x + b1) on DVE: (in + b1) max 0
                        nc.vector.tensor_scalar(
                            out=hT[:, fc, :], in0=ps_h,
                            scalar1=b1_sb[:, fc:fc + 1], scalar2=0.0,
                            op0=ALU.add, op1=ALU.max)
                    rv = rsi[b][t // 2][:]
                    for ec in range(KC):
                        ps_y = pp.tile([128, 512], F32, tag="pp",
                                       name="ps_y")
                        for fc in range(FCL):
                            nc.tensor.matmul(ps_y, w2[:, fc, ts(ec, 128)],
                                             hT[:, fc, :],
                                             start=(fc == 0),
                                             stop=(fc == FCL - 1))
                        ys = sb.tile([128, 512], F16, tag="ys", bufs=3,
                                     name="ys")
                        nc.vector.tensor_copy(out=ys, in_=ps_y)
                        nc.sync.dma_start(out=rv[ec, :, ts(t % 2, 512)],
                                          in_=ys)
                    if t % 2 == 1:
                        nc.gpsimd.collective_compute(
                            "ReduceScatter", ALU.add, replica_groups=RG,
                            ins=[rsi[b][t // 2][:]],
                            outs=[rso[b][t // 2][:]])

            # ======= y + b2 + x2 residual, seqnorm, write out (per b) ======
            # per-quarter assembly + stats so only the last RS is exposed
            x3 = sb.tile([128, B, S], F32, tag="xl", bufs=2, name="x3")
            for b in range(B):
                stats = sb.tile([128, ST, 6], F32, tag="bnst", bufs=2,
                                name="stats")
                for half in range(2):
                    yh = sb.tile([128, SH], F16, tag="yh", bufs=2,
                                 name="yh")
                    nc.sync.dma_start(out=yh, in_=rso[b][half][:])
                    nc.vector.scalar_tensor_tensor(
                        out=x3[:, b, ts(half, SH)], in0=yh,
                        scalar=b2_sb[:, 0:1], op0=ALU.add,
                        in1=x2[:, b, ts(half, SH)], op1=ALU.add)
                    for g in range(2):
                        nc.vector.bn_stats(
                            out=stats[:, 2 * half + g, :],
                            in_=x3[:, b, ts(2 * half + g, 512)])
                mv = sb.tile([128, 2], F32, tag="bnmv", bufs=2, name="mv")
                nc.vector.bn_aggr(out=mv, in_=stats)
                r = sb.tile([128, 1], F32, tag="bnr", bufs=2, name="r")
                nc.vector.reciprocal(r, mv[:, 1:2])
                nc.vector.tensor_scalar(out=r, in0=r, scalar1=float(VARF),
                                        scalar2=None, op0=ALU.mult)
                mr = sb.tile([128, 1], F32, tag="bnmr", bufs=2, name="mr")
                nc.vector.scalar_tensor_tensor(
                    out=mr, in0=mv[:, 0:1], scalar=-1.0, op0=ALU.mult,
                    in1=r, op1=ALU.mult)
                nc.vector.scalar_tensor_tensor(
                    out=x3[:, b, :], in0=x3[:, b, :], scalar=r,
                    op0=ALU.mult, in1=mr.to_broadcast((128, S)),
                    op1=ALU.add)
                nc.sync.dma_start(out=bview(outT)[:, b, :], in_=x3[:, b, :])

    nc.compile()
    return nc


def _get_nc():
    global _CACHED_NC
    if _CACHED_NC is None:
        _CACHED_NC = _build()
    return _CACHED_NC


def _chunked(a):
    """[D, N] -> [128, D//128, N] with [p, c, n] = a[128c+p, n]."""
    d, n = a.shape
    return np.ascontiguousarray(
        a.reshape(d // 128, 128, n).transpose(1, 0, 2).astype(np.float16))


def _make_in_maps(decoder_input, encode_input,
                  Wq1, Wk1, Wv1, bq1, bk1, bv1,
                  Wq2, Wk2, Wv2, bq2, bk2, bv2,
                  W1, b1, W2, b2):
    xT = np.ascontiguousarray(
        np.transpose(np.asarray(decoder_input, np.float32), (0, 2, 1)))
    eT = np.transpose(np.asarray(encode_input, np.float32), (0, 2, 1))
    # [128, B, KC, S] fp16
    xTd_all = np.ascontiguousarray(
        xT.reshape(B, KC, 128, S).transpose(2, 0, 1, 3).astype(np.float16))
    encd_all = np.ascontiguousarray(
        eT.reshape(B, KC, 128, S).transpose(2, 0, 1, 3).astype(np.float16))
    in_maps = []
    for r in range(NCORES):
        hs = slice(DL * r, DL * (r + 1))
        fs = slice(FFL * r, FFL * (r + 1))
        resd = np.ascontiguousarray(
            xT[:, hs, :].transpose(1, 0, 2).astype(np.float16))
        bqk_arr = np.stack([bq1[hs], bk1[hs], bq2[hs], bk2[hs]],
                           axis=1).astype(np.float32)  # [128, 4]
        bv_arr = np.concatenate([
            bv1[hs].reshape(HL, DK).T, bv2[hs].reshape(HL, DK).T,
        ], axis=1).astype(np.float32)                  # [64, 4]
        bv_arr = np.concatenate([bv_arr, bv_arr], axis=0)  # [128, 4]
        in_maps.append({
            "xTd": xTd_all,
            "encd": encd_all,
            "resd": resd,
            "wq1d": _chunked(np.ascontiguousarray(Wq1[:, hs])),
            "wk1d": _chunked(np.ascontiguousarray(Wk1[:, hs])),
            "wv1d": _chunked(np.ascontiguousarray(Wv1[:, hs])),
            "wq2d": _chunked(np.ascontiguousarray(Wq2[:, hs])),
            "wk2d": _chunked(np.ascontiguousarray(Wk2[:, hs])),
            "wv2d": _chunked(np.ascontiguousarray(Wv2[:, hs])),
            "w1d": _chunked(np.ascontiguousarray(W1[:, fs])),
            "w2d": _chunked(np.ascontiguousarray(W2[fs, :])),
            "bqkd": bqk_arr,
            "bvd": bv_arr,
            "b1d": np.ascontiguousarray(
                b1[fs].reshape(FCL, 128).T.astype(np.float32)),
            "b2d": np.ascontiguousarray(
                b2[hs].reshape(128, 1).astype(np.float32)),
        })
    return in_maps


def kernel(**inputs):
    nc = _get_nc()
    in_maps = _make_in_maps(**{k: np.asarray(v) for k, v in inputs.items()})
    res = run_bass_kernel_spmd(nc, in_maps, core_ids=list(range(NCORES)),
                               trace=False)
    out = np.empty((B, S, D), np.float32)
    for r in range(NCORES):
        hs = slice(DL * r, DL * (r + 1))
        o = res.results[r]["outT"]                     # [B*DL, S]
        for b in range(B):
            out[b, :, hs] = o[b * DL:(b + 1) * DL].T
    return out


# revision 35
# speedup vs baseline: 1.0558x; 1.0049x over previous
"""Trainium2 Bass kernel for a 3-sublayer decoder block (nn_DecoderLayer).

Reference computation (B=2, S=2048, D=1024, H=16, DK=64, FF=4096, fp32):
  sa = causal_mha(x, x)          ; x1 = seqnorm(sa + x)
  ca = mha(x1, enc)              ; x2 = seqnorm(ca + x1)
  ffn = relu(x2 @ W1 + b1) @ W2 + b2 ; out = seqnorm(ffn + x2)
seqnorm normalizes over the SEQUENCE dim and divides by the unbiased VARIANCE
(reference quirk); attention has no output projection.

Sharding (8 cores, one replica group): every core processes BOTH batch
elements; heads split 2-per-core (a 128-wide channel slice of every sublayer
output per batch stays fully local, incl. seqnorm); FF hidden split
512-per-core (Megatron column/row). Collectives: AllGather(x1) and
AllGather(x2) per BATCH element (so AG(b0) hides under attention compute of
b1), ReduceScatter of FFN partials per (batch, seq-half) (so only the last
quarter-ish is tail-exposed).

v2 vs the fp32r baseline (1.44 ms):
 * fp16 operands end-to-end (host-cast): halves HBM+collective bytes, enables
   FWL fast weight loads (fp32 LDWEIGHTS was 423us of serialized PE time),
   and drops PE power (the fp32 version sat at K=4/8 = 1.2 GHz throttle).
 * per-batch collective pipelining (above) removes the ~310us of dead zones.
 * big streaming loads on the sync (HWDGE) queue, collective bounce stores as
   casting SWDGE DMAs on the gpsimd queue - so enc/x1f/x2f prefetch is never
   stuck behind an AllGather in the same in-order queue.
 * softmax denominator reciprocal via reciprocal_approx_fast (custom DVE op,
   ~5x faster than the iterative divide: that was 107us of DVE time).
 * PSUM: score tiles double-buffered (2x[128,1024]) so scores(c+1) overlaps
   exp(c) on ACT; PV accumulates both heads into one [128,1024] bank pair
   drained in a single copy.
All activations transposed on-chip ([d, s]); E^T = exp(K @ Q^T) computed
directly; softmax denominators from ones-columns appended to V.
"""

import os
import sys

import numpy as np

for _p in ("/opt/trn_rl_repo", "/root/.axon_site/_ro/trn_rl_repo"):
    if _p not in sys.path and os.path.isdir(_p):
        sys.path.append(_p)

import concourse.bass as bass
import concourse.mybir as mybir
import concourse.tile as tile
from concourse import bacc
from concourse.bass import ts
from concourse.bass_utils import run_bass_kernel_spmd

F32 = mybir.dt.float32
F16 = mybir.dt.float16
AF = mybir.ActivationFunctionType
ALU = mybir.AluOpType

B, S, D, H = 2, 2048, 1024, 16
DK = D // H            # 64
FF = 4 * D             # 4096
NCORES = 8
HL = H // NCORES       # 2 heads per core
DL = DK * HL           # 128 channels per core
FFL = FF // NCORES     # 512 ff dims per core
KC = D // 128          # 8 contraction chunks of the full model dim
FCL = FFL // 128       # 4 local ff chunks
ST = S // 512          # 4 sequence tiles of 512
SC = S // 128          # 16 sequence chunks of 128
SH = S // 2            # sequence half
SCALE = 1.0 / np.sqrt(DK)
VARF = (S - 1) / S     # unbiased-variance factor applied to 1/var_pop
VW = DK + 1            # 65: per-head stationary width in vO (v + ones col)

RG = [[0, 1, 2, 3, 4, 5, 6, 7]]

_CACHED_NC = None


def _build():
    nc = bacc.Bacc("TRN2", target_bir_lowering=False, debug=False,
                   num_devices=NCORES)

    # ---- per-core external inputs (fp16 except small fp32 biases) ----
    xTd = nc.dram_tensor("xTd", [128, B, KC, S], F16, kind="ExternalInput")
    encd = nc.dram_tensor("encd", [128, B, KC, S], F16, kind="ExternalInput")
    resd = nc.dram_tensor("resd", [128, B, S], F16, kind="ExternalInput")
    wq1d = nc.dram_tensor("wq1d", [128, KC, DL], F16, kind="ExternalInput")
    wk1d = nc.dram_tensor("wk1d", [128, KC, DL], F16, kind="ExternalInput")
    wv1d = nc.dram_tensor("wv1d", [128, KC, DL], F16, kind="ExternalInput")
    wq2d = nc.dram_tensor("wq2d", [128, KC, DL], F16, kind="ExternalInput")
    wk2d = nc.dram_tensor("wk2d", [128, KC, DL], F16, kind="ExternalInput")
    wv2d = nc.dram_tensor("wv2d", [128, KC, DL], F16, kind="ExternalInput")
    w1d = nc.dram_tensor("w1d", [128, KC, FFL], F16, kind="ExternalInput")
    w2d = nc.dram_tensor("w2d", [128, FCL, D], F16, kind="ExternalInput")
    bqkd = nc.dram_tensor("bqkd", [128, 4], F32, kind="ExternalInput")
    # bv values per (head, dk); rows 64:128 duplicate rows 0:64 so the h1
    # stt can use a partition-64-based scalar AP
    bvd = nc.dram_tensor("bvd", [128, 2 * HL], F32, kind="ExternalInput")
    b1d = nc.dram_tensor("b1d", [128, FCL], F32, kind="ExternalInput")
    b2d = nc.dram_tensor("b2d", [128, 1], F32, kind="ExternalInput")

    outT = nc.dram_tensor("outT", [B * DL, S], F32, kind="ExternalOutput")
    DBG = bool(os.environ.get("BASSDBG"))
    if DBG:
        dbg_x1 = nc.dram_tensor("dbg_x1", [B * DL, S], F32,
                                kind="ExternalOutput")
        dbg_x2 = nc.dram_tensor("dbg_x2", [B * DL, S], F32,
                                kind="ExternalOutput")
        dbg_k1 = nc.dram_tensor("dbg_k1", [128, B, S], F16,
                                kind="ExternalOutput")
        dbg_q1 = nc.dram_tensor("dbg_q1", [128, B, ST, 512], F16,
                                kind="ExternalOutput")
        dbg_v1 = nc.dram_tensor("dbg_v1", [128, B, SC, 2 * VW], F16,
                                kind="ExternalOutput")
        dbg_pre = nc.dram_tensor("dbg_pre", [128, B, S], F32,
                                 kind="ExternalOutput")

    def bview(t):   # [B*128, s] -> [p, b, s]
        return t[:].rearrange("(b p) s -> p b s", p=128)

    with tile.TileContext(nc) as tc:
        import contextlib
        ctx = contextlib.ExitStack()
        with ctx:
            sb = ctx.enter_context(tc.tile_pool(name="sb", bufs=1))
            dram = ctx.enter_context(tc.tile_pool(name="dr", bufs=1,
                                                  space="DRAM"))
            pp = ctx.enter_context(tc.tile_pool(name="pp", bufs=2,
                                                space="PSUM"))
            pe = ctx.enter_context(tc.tile_pool(name="pe", bufs=2,
                                                space="PSUM"))
            pz = ctx.enter_context(tc.tile_pool(name="pz", bufs=1,
                                                space="PSUM"))

            # ---- collective bounce buffers (per batch element) ----
            x1b = [dram.tile([DL, S], F16, tag=f"x1b{b}", name=f"x1b{b}")
                   for b in range(B)]
            x1f = [dram.tile([NCORES, DL, S], F16, tag=f"x1f{b}",
                             name=f"x1f{b}", addr_space="Shared")
                   for b in range(B)]
            x2b = [dram.tile([DL, S], F16, tag=f"x2b{b}", name=f"x2b{b}")
                   for b in range(B)]
            x2f = [dram.tile([NCORES, DL, S], F16, tag=f"x2f{b}",
                             name=f"x2f{b}", addr_space="Shared")
                   for b in range(B)]
            # RS per (b, seq-half): few enough that the serial cc stream
            # doesn't congest, small enough that only ~one is tail-exposed
            rsi = [[dram.tile([NCORES, DL, SH], F16, tag=f"rsi{b}{h}",
                              name=f"rsi{b}{h}") for h in range(2)]
                   for b in range(B)]
            rso = [[dram.tile([DL, SH], F16, tag=f"rso{b}{h}",
                              name=f"rso{b}{h}")
                    for h in range(2)] for b in range(B)]
            # dummy collective issued at t=0: the first collective on the
            # device carries the rank-sync barrier (~40-50us of launch skew);
            # firing it up front hides that under the warmup DMAs/compute
            dumb = dram.tile([128, 2], F16, tag="dumb", name="dumb")
            dumf = dram.tile([NCORES, 128, 2], F16, tag="dumf", name="dumf",
                             addr_space="Shared")
            nc.gpsimd.collective_compute(
                "AllGather", ALU.bypass, replica_groups=RG,
                ins=[dumb[:]], outs=[dumf[:]])

            # ---- small persistent tiles ----
            bqk_sb = sb.tile([128, 4], F32, tag="bias", bufs=1)
            nc.sync.dma_start(out=bqk_sb, in_=bqkd[:])
            bv_sb = sb.tile([128, 2 * HL], F32, tag="bias2", bufs=1)
            nc.sync.dma_start(out=bv_sb, in_=bvd[:])
            b1_sb = sb.tile([128, FCL], F32, tag="bias3", bufs=1)
            nc.sync.dma_start(out=b1_sb, in_=b1d[:])
            b2_sb = sb.tile([128, 1], F32, tag="bias4", bufs=1)
            nc.sync.dma_start(out=b2_sb, in_=b2d[:])

            # identity for PE transposes (fp16)
            id16 = sb.tile([128, 128], F16, tag="id16", bufs=1)
            nc.vector.memset(id16, 1.0)
            nc.gpsimd.affine_select(out=id16, in_=id16,
                                    compare_op=ALU.is_equal, fill=0.0,
                                    base=0, channel_multiplier=-1,
                                    pattern=[[1, 128]])

            # ---- weights (all loaded up front; ~3 MB fp16) ----
            def load_w(dram_t, name, cols, tag):
                w = sb.tile([128, KC, cols], F16, tag=tag, bufs=1, name=name)
                nc.sync.dma_start(out=w, in_=dram_t[:])
                return w

            # only sublayer-1 weights now; the rest are loaded after the
            # qkv1 xs loads are queued so the first matmul starts early
            wq1 = load_w(wq1d, "wq1", DL, "wqkv")
            wk1 = load_w(wk1d, "wk1", DL, "wqkv2")
            wv1 = load_w(wv1d, "wv1", DL, "wqkv3")

            # ---- persistent activation tiles ----
            kT1 = sb.tile([128, B, S], F16, tag="kT1", bufs=1, name="kT1")
            kT2 = sb.tile([128, B, S], F16, tag="kT2", bufs=1, name="kT2")
            # vO layout: [h0 ch(64) | ones | h1 ch(64) | ones] => stationary
            # for head h is the contiguous [128, 65] slice at 65*h.
            vO1 = sb.tile([128, B, SC, 2 * VW], F16, tag="vO1", bufs=1,
                          name="vO1")
            vO2 = sb.tile([128, B, SC, 2 * VW], F16, tag="vO2", bufs=1,
                          name="vO2")
            for vO in (vO1, vO2):
                nc.vector.memset(vO[:, :, :, DK:DK + 1], 1.0)
                nc.vector.memset(vO[:, :, :, DK + VW:DK + VW + 1], 1.0)
            qt1 = sb.tile([128, B, ST, 512], F16, tag="qt", bufs=2,
                          name="qt1")
            qt2 = sb.tile([128, B, ST, 512], F16, tag="qt", bufs=2,
                          name="qt2")
            x1 = sb.tile([128, B, S], F32, tag="xl", bufs=2, name="x1")
            x2 = sb.tile([128, B, S], F32, tag="xl", bufs=2, name="x2")

            def proj128(xs, w, bias_col, out_ap):
                """One [128, 512] projection: out = W.T @ x + bias (ACT
                drain psum->sbuf fp16 with per-channel bias)."""
                ps = pp.tile([128, 512], F32, tag="pp", name="ps")
                for k in range(KC):
                    nc.tensor.matmul(ps, w[:, k, :], xs[:, k, :],
                                     start=(k == 0), stop=(k == KC - 1))
                # drain on DVE: ACT is the bottleneck engine in attention
                nc.vector.tensor_scalar(
                    out=out_ap, in0=ps,
                    scalar1=bqk_sb[:, bias_col:bias_col + 1],
                    scalar2=None, op0=ALU.add)

            def load_xs(src_ap):
                xs = sb.tile([128, KC, 512], F16, tag="xs", bufs=4,
                             name="xs")
                nc.sync.dma_start(out=xs, in_=src_ap)
                return xs

            def qkv_tile(xs, b, t, wq, wk, wv, kT, vO, qt, qcol, kcol,
                         bv_off):
                """One (b, 512-seq-tile): q/k transposed projections, v
                projected then flipped back via PE transposes into vO (ones
                columns persist from the initial memset)."""
                if wq is not None:
                    proj128(xs, wq, qcol, qt[:, b, t, :])
                proj128(xs, wk, kcol, kT[:, b, ts(t, 512)])
                # vT [128(2h*dk), 512] (+bias) then transpose per 128-block
                ps = pp.tile([128, 512], F32, tag="pp", name="ps")
                for k in range(KC):
                    nc.tensor.matmul(ps, wv[:, k, :], xs[:, k, :],
                                     start=(k == 0), stop=(k == KC - 1))
                # no bias here: bv is added after softmax-normalize (rows of
                # P sum to 1, so PV(v)+bv == P(v+bv) normalized)
                vt = sb.tile([128, 512], F16, tag="vt", bufs=2, name="vt")
                nc.vector.tensor_copy(out=vt, in_=ps)
                tp = pp.tile([128, 512], F16, tag="pp", name="tp")
                for sc in range(4):
                    nc.tensor.transpose(tp[:, ts(sc, 128)],
                                        vt[:, ts(sc, 128)], id16)
                # one strided copy: [sc, h, dk] -> vO cols {0:64, 65:129}
                nc.vector.tensor_copy(
                    out=vO[:, b, 4 * t:4 * t + 4, :]
                        .rearrange("p c (h w) -> p c h w", h=2)[:, :, :, 0:DK],
                    in_=tp[:, :].rearrange("p (c h w) -> p c h w", c=4, h=2))

            def attn_tile(b, t, qt, kT, vO, xout, resid_ap, bv_off, causal):
                """One (b, sq-tile): per sk-chunk, E^T for both heads in one
                2-bank psum (disjoint PE row groups -> concurrent), exp on
                ACT (fp16 out), causal mask on gpsimd, PV for both heads into
                one 2-bank psum; then one drain + approx-reciprocal denom +
                normalize + bias + residual into xout."""
                nchunks = (4 * t + 4) if causal else SC
                zps = pz.tile([128, 1024], F32, tag="pz", name="zps")
                for c in range(nchunks):
                    eps = pe.tile([128, 1024], F32, tag="pe", name="eps")
                    for h in range(HL):
                        hb = h * 64
                        nc.tensor.matmul(eps[:, ts(h, 512)],
                                         kT[hb:hb + 64, b, ts(c, 128)],
                                         qt[hb:hb + 64, b, t, :],
                                         start=True, stop=True)
                    et = sb.tile([128, 1024], F16, tag="E", bufs=3,
                                 name="et")
                    nc.scalar.activation(out=et, in_=eps, func=AF.Exp,
                                         scale=float(SCALE))
                    if causal and c >= 4 * t:
                        j = c - 4 * t
                        for h in range(HL):
                            nc.gpsimd.affine_select(
                                out=et[:, ts(h, 512)],
                                in_=et[:, ts(h, 512)],
                                compare_op=ALU.is_ge,
                                fill=0.0, base=-(j * 128),
                                channel_multiplier=-1,
                                pattern=[[1, 512]])
                    for h in range(HL):
                        nc.tensor.matmul(
                            zps[0:VW, ts(h, 512)],
                            vO[:, b, c, VW * h:VW * h + VW],
                            et[:, ts(h, 512)],
                            start=(c == 0), stop=(c == nchunks - 1))
                # drain [65, 1024] once; row 64 = softmax denominators
                zsb = sb.tile([VW, 1024], F32, tag="zsb", bufs=2, name="zsb")
                nc.vector.tensor_copy(out=zsb, in_=zps[0:VW, :])
                dr = sb.tile([1, 1024], F32, tag="dr", bufs=2, name="dr")
                nc.vector.tensor_copy(out=dr, in_=zsb[DK:DK + 1, :])
                rb = sb.tile([64, 1024], F32, tag="rb", bufs=2, name="rb")
                nc.gpsimd.partition_broadcast(out_ap=rb, in_ap=dr)
                nc.vector.reciprocal_approx_fast(out=rb, in_=rb)
                zn = sb.tile([64, 1024], F32, tag="zn", bufs=2, name="zn")
                nc.vector.tensor_mul(zn, zsb[0:DK, :], rb)
                # h0 writes in place; h1 needs a partition shift (stt operands
                # must share a start partition; only copies may shift)
                nc.vector.scalar_tensor_tensor(
                    out=xout[0:64, b, ts(t, 512)],
                    in0=zn[:, 0:512],
                    scalar=bv_sb[0:64, bv_off:bv_off + 1], op0=ALU.add,
                    in1=resid_ap[0:64, b, ts(t, 512)], op1=ALU.add)
                nc.vector.tensor_copy(out=xout[64:128, b, ts(t, 512)],
                                      in_=zn[:, 512:1024])
                nc.vector.scalar_tensor_tensor(
                    out=xout[64:128, b, ts(t, 512)],
                    in0=xout[64:128, b, ts(t, 512)],
                    scalar=bv_sb[64:128, bv_off + 1:bv_off + 2], op0=ALU.add,
                    in1=resid_ap[64:128, b, ts(t, 512)], op1=ALU.add)

            def seqnorm_b(xt, b):
                """Sequence-norm of [128, S] f32 (divide by unbiased var)."""
                stats = sb.tile([128, ST, 6], F32, tag="bnst", bufs=2,
                                name="stats")
                for g in range(ST):
                    nc.vector.bn_stats(out=stats[:, g, :],
                                       in_=xt[:, b, ts(g, 512)])
                mv = sb.tile([128, 2], F32, tag="bnmv", bufs=2, name="mv")
                nc.vector.bn_aggr(out=mv, in_=stats)
                r = sb.tile([128, 1], F32, tag="bnr", bufs=2, name="r")
                nc.vector.reciprocal(r, mv[:, 1:2])
                nc.vector.tensor_scalar(out=r, in0=r, scalar1=float(VARF),
                                        scalar2=None, op0=ALU.mult)
                mr = sb.tile([128, 1], F32, tag="bnmr", bufs=2, name="mr")
                nc.vector.scalar_tensor_tensor(
                    out=mr, in0=mv[:, 0:1], scalar=-1.0, op0=ALU.mult,
                    in1=r, op1=ALU.mult)
                nc.vector.scalar_tensor_tensor(
                    out=xt[:, b, :], in0=xt[:, b, :], scalar=r,
                    op0=ALU.mult, in1=mr.to_broadcast((128, S)),
                    op1=ALU.add)

            def ag_issue(xt, b, bb, fb):
                """Cast+store x[:, b, :] to the bounce buffer (SWDGE casting
                DMA on the gpsimd queue) and trigger the AllGather."""
                nc.gpsimd.dma_start(out=bb[:], in_=xt[:, b, :])
                nc.gpsimd.collective_compute(
                    "AllGather", ALU.bypass, replica_groups=RG,
                    ins=[bb[:]], outs=[fb[:]])

            # ================= sublayer 1: causal self-attention ===========
            for b in range(B):
                for t in range(ST):
                    xs = load_xs(xTd.ap()[:, b, :, ts(t, 512)])
                    qkv_tile(xs, b, t, wq1, wk1, wv1, kT1, vO1, qt1,
                             qcol=0, kcol=1, bv_off=0)
            # deferred loads: residual + later-phase weights queue behind
            # the qkv1 xs loads
            resid = sb.tile([128, B, S], F16, tag="res", bufs=1,
                            name="resid")
            nc.sync.dma_start(out=resid, in_=resd[:])
            wq2 = load_w(wq2d, "wq2", DL, "wqkv4")
            wk2 = load_w(wk2d, "wk2", DL, "wqkv5")
            wv2 = load_w(wv2d, "wv2", DL, "wqkv6")
            w1 = sb.tile([128, KC, FFL], F16, tag="w1", bufs=1, name="w1")
            nc.sync.dma_start(out=w1, in_=w1d[:])
            w2 = sb.tile([128, FCL, D], F16, tag="w2", bufs=1, name="w2")
            nc.sync.dma_start(out=w2, in_=w2d[:])
            # prefetch enc tiles now (sync queue, streams during attn1)
            enc_xs = [[load_xs(encd.ap()[:, b, :, ts(t, 512)])
                       for t in range(ST)] for b in range(B)]
            q2_xs = [None] * B
            for b in range(B):
                for t in range(ST):
                    attn_tile(b, t, qt1, kT1, vO1, x1, resid, bv_off=0,
                              causal=True)
                if DBG:
                    nc.sync.dma_start(out=dbg_pre[:, b, :], in_=x1[:, b, :])
                seqnorm_b(x1, b)
                ag_issue(x1, b, x1b[b], x1f[b])
                # q2 loads for this b (gated on AG1(b); behind enc loads in
                # the sync queue, so enc prefetch is never blocked)
                x1f_v = x1f[b][:].rearrange("r p s -> p r s")
                q2_xs[b] = [load_xs(x1f_v[:, :, ts(t, 512)])
                            for t in range(ST)]
                # fill attn1(b)'s ACT-bound lag with enc K/V projections
                for t in range(ST):
                    qkv_tile(enc_xs[b][t], b, t, None, wk2, wv2, kT2, vO2,
                             None, qcol=None, kcol=3, bv_off=HL)
            if DBG:
                for b in range(B):
                    nc.sync.dma_start(out=bview(dbg_x1)[:, b, :],
                                      in_=x1[:, b, :])
                nc.sync.dma_start(out=dbg_k1[:], in_=kT1)
                nc.sync.dma_start(out=dbg_q1[:], in_=qt1)
                nc.sync.dma_start(out=dbg_v1[:], in_=vO1)

            # ================= sublayer 2: cross-attention =================
            # q2 projection interleaved per tile so ACT's exp stream starts
            # after a single projection instead of all eight
            for b in range(B):
                for t in range(ST):
                    proj128(q2_xs[b][t], wq2, 2, qt2[:, b, t, :])
                    attn_tile(b, t, qt2, kT2, vO2, x2, x1, bv_off=HL,
                              causal=False)
                seqnorm_b(x2, b)
                ag_issue(x2, b, x2b[b], x2f[b])
            if DBG:
                for b in range(B):
                    nc.sync.dma_start(out=bview(dbg_x2)[:, b, :],
                                      in_=x2[:, b, :])

            # ================= sublayer 3: FFN =============================
            for b in range(B):
                x2f_v = x2f[b][:].rearrange("r p s -> p r s")
                for t in range(ST):
                    xs = load_xs(x2f_v[:, :, ts(t, 512)])
                    hT = sb.tile([128, FCL, 512], F16, tag="hT", bufs=2,
                                 name="hT")
                    for fc in range(FCL):
                        ps_h = pp.tile([128, 512], F32, tag="pp",
                                       name="ps_h")
                        for k in range(KC):
                            nc.tensor.matmul(ps_h, w1[:, k, ts(fc, 128)],
                                             xs[:, k, :],
                                             start=(k == 0),
                                             stop=(k == KC - 1))
                        # relu(x + b1) on DVE: (in + b1) max 0
                        nc.vector.tensor_scalar(
                            out=hT[:, fc, :], in0=ps_h,
                            scalar1=b1_sb[:, fc:fc + 1], scalar2=0.0,
                            op0=ALU.add, op1=ALU.max)
                    rv = rsi[b][t // 2][:]
                    for ec in range(KC):
                        ps_y = pp.tile([128, 512], F32, tag="pp",
                                       name="ps_y")
                        for fc in range(FCL):
                            nc.tensor.matmul(ps_y, w2[:, fc, ts(ec, 128)],
                                             hT[:, fc, :],
                                             start=(fc == 0),
                                             stop=(fc == FCL - 1))
                        ys = sb.tile([128, 512], F16, tag="ys", bufs=3,
                                     name="ys")
                        nc.vector.tensor_copy(out=ys, in_=ps_y)
                        nc.sync.dma_start(out=rv[ec, :, ts(t % 2, 512)],
                                          in_=ys)
                    if t % 2 == 1:
                        nc.gpsimd.collective_compute(
                            "ReduceScatter", ALU.add, replica_groups=RG,
                            ins=[rsi[b][t // 2][:]],
                            outs=[rso[b][t // 2][:]])

            # ======= y + b2 + x2 residual, seqnorm, write out (per b) ======
            # negative high_priority offset = LOW priority: these ops wait on
            # ReduceScatter results, and the scheduler (whose collective cost
            # model is optimistic) otherwise hoists them into the middle of
            # the FFN's DVE stream, stalling the whole queue on the RS
            ctx.enter_context(tc.high_priority(offset=-1000000))
            x3 = sb.tile([128, B, S], F32, tag="xl", bufs=2, name="x3")
            for b in range(B):
                stats = sb.tile([128, ST, 6], F32, tag="bnst", bufs=2,
                                name="stats")
                for half in range(2):
                    yh = sb.tile([128, SH], F16, tag="yh", bufs=2,
                                 name="yh")
                    nc.sync.dma_start(out=yh, in_=rso[b][half][:])
                    nc.vector.scalar_tensor_tensor(
                        out=x3[:, b, ts(half, SH)], in0=yh,
                        scalar=b2_sb[:, 0:1], op0=ALU.add,
                        in1=x2[:, b, ts(half, SH)], op1=ALU.add)
                    for g in range(2):
                        nc.vector.bn_stats(
                            out=stats[:, 2 * half + g, :],
                            in_=x3[:, b, ts(2 * half + g, 512)])
                mv = sb.tile([128, 2], F32, tag="bnmv", bufs=2, name="mv")
                nc.vector.bn_aggr(out=mv, in_=stats)
                r = sb.tile([128, 1], F32, tag="bnr", bufs=2, name="r")
                nc.vector.reciprocal(r, mv[:, 1:2])
                nc.vector.tensor_scalar(out=r, in0=r, scalar1=float(VARF),
                                        scalar2=None, op0=ALU.mult)
                mr = sb.tile([128, 1], F32, tag="bnmr", bufs=2, name="mr")
                nc.vector.scalar_tensor_tensor(
                    out=mr, in0=mv[:, 0:1], scalar=-1.0, op0=ALU.mult,
                    in1=r, op1=ALU.mult)
                nc.vector.scalar_tensor_tensor(
                    out=x3[:, b, :], in0=x3[:, b, :], scalar=r,
                    op0=ALU.mult, in1=mr.to_broadcast((128, S)),
                    op1=ALU.add)
                nc.sync.dma_start(out=bview(outT)[:, b, :], in_=x3[:, b, :])

    nc.compile()
    return nc


def _get_nc():
    global _CACHED_NC
    if _CACHED_NC is None:
        _CACHED_NC = _build()
    return _CACHED_NC


def _chunked(a):
    """[D, N] -> [128, D//128, N] with [p, c, n] = a[128c+p, n]."""
    d, n = a.shape
    return np.ascontiguousarray(
        a.reshape(d // 128, 128, n).transpose(1, 0, 2).astype(np.float16))


def _make_in_maps(decoder_input, encode_input,
                  Wq1, Wk1, Wv1, bq1, bk1, bv1,
                  Wq2, Wk2, Wv2, bq2, bk2, bv2,
                  W1, b1, W2, b2):
    xT = np.ascontiguousarray(
        np.transpose(np.asarray(decoder_input, np.float32), (0, 2, 1)))
    eT = np.transpose(np.asarray(encode_input, np.float32), (0, 2, 1))
    # [128, B, KC, S] fp16
    xTd_all = np.ascontiguousarray(
        xT.reshape(B, KC, 128, S).transpose(2, 0, 1, 3).astype(np.float16))
    encd_all = np.ascontiguousarray(
        eT.reshape(B, KC, 128, S).transpose(2, 0, 1, 3).astype(np.float16))
    in_maps = []
    for r in range(NCORES):
        hs = slice(DL * r, DL * (r + 1))
        fs = slice(FFL * r, FFL * (r + 1))
        resd = np.ascontiguousarray(
            xT[:, hs, :].transpose(1, 0, 2).astype(np.float16))
        bqk_arr = np.stack([bq1[hs], bk1[hs], bq2[hs], bk2[hs]],
                           axis=1).astype(np.float32)  # [128, 4]
        bv_arr = np.concatenate([
            bv1[hs].reshape(HL, DK).T, bv2[hs].reshape(HL, DK).T,
        ], axis=1).astype(np.float32)                  # [64, 4]
        bv_arr = np.concatenate([bv_arr, bv_arr], axis=0)  # [128, 4]
        in_maps.append({
            "xTd": xTd_all,
            "encd": encd_all,
            "resd": resd,
            "wq1d": _chunked(np.ascontiguousarray(Wq1[:, hs])),
            "wk1d": _chunked(np.ascontiguousarray(Wk1[:, hs])),
            "wv1d": _chunked(np.ascontiguousarray(Wv1[:, hs])),
            "wq2d": _chunked(np.ascontiguousarray(Wq2[:, hs])),
            "wk2d": _chunked(np.ascontiguousarray(Wk2[:, hs])),
            "wv2d": _chunked(np.ascontiguousarray(Wv2[:, hs])),
            "w1d": _chunked(np.ascontiguousarray(W1[:, fs])),
            "w2d": _chunked(np.ascontiguousarray(W2[fs, :])),
            "bqkd": bqk_arr,
            "bvd": bv_arr,
            "b1d": np.ascontiguousarray(
                b1[fs].reshape(FCL, 128).T.astype(np.float32)),
            "b2d": np.ascontiguousarray(
                b2[hs].reshape(128, 1).astype(np.float32)),
        })
    return in_maps


def kernel(**inputs):
    nc = _get_nc()
    in_maps = _make_in_maps(**{k: np.asarray(v) for k, v in inputs.items()})
    res = run_bass_kernel_spmd(nc, in_maps, core_ids=list(range(NCORES)),
                               trace=False)
    out = np.empty((B, S, D), np.float32)
    for r in range(NCORES):
        hs = slice(DL * r, DL * (r + 1))
        o = res.results[r]["outT"]                     # [B*DL, S]
        for b in range(B):
            out[b, :, hs] = o[b * DL:(b + 1) * DL].T
    return out


# revision 36
# speedup vs baseline: 1.0908x; 1.0331x over previous
"""Trainium2 Bass kernel for a 3-sublayer decoder block (nn_DecoderLayer).

Reference computation (B=2, S=2048, D=1024, H=16, DK=64, FF=4096, fp32):
  sa = causal_mha(x, x)          ; x1 = seqnorm(sa + x)
  ca = mha(x1, enc)              ; x2 = seqnorm(ca + x1)
  ffn = relu(x2 @ W1 + b1) @ W2 + b2 ; out = seqnorm(ffn + x2)
seqnorm normalizes over the SEQUENCE dim and divides by the unbiased VARIANCE
(reference quirk); attention has no output projection.

Sharding (8 cores, one replica group): every core processes BOTH batch
elements; heads split 2-per-core (a 128-wide channel slice of every sublayer
output per batch stays fully local, incl. seqnorm); FF hidden split
512-per-core (Megatron column/row). Collectives: AllGather(x1) and
AllGather(x2) per BATCH element (so AG(b0) hides under attention compute of
b1), ReduceScatter of FFN partials per (batch, seq-half) (so only the last
quarter-ish is tail-exposed).

v2 vs the fp32r baseline (1.44 ms):
 * fp16 operands end-to-end (host-cast): halves HBM+collective bytes, enables
   FWL fast weight loads (fp32 LDWEIGHTS was 423us of serialized PE time),
   and drops PE power (the fp32 version sat at K=4/8 = 1.2 GHz throttle).
 * per-batch collective pipelining (above) removes the ~310us of dead zones.
 * big streaming loads on the sync (HWDGE) queue, collective bounce stores as
   casting SWDGE DMAs on the gpsimd queue - so enc/x1f/x2f prefetch is never
   stuck behind an AllGather in the same in-order queue.
 * softmax denominator reciprocal via reciprocal_approx_fast (custom DVE op,
   ~5x faster than the iterative divide: that was 107us of DVE time).
 * PSUM: score tiles double-buffered (2x[128,1024]) so scores(c+1) overlaps
   exp(c) on ACT; PV accumulates both heads into one [128,1024] bank pair
   drained in a single copy.
All activations transposed on-chip ([d, s]); E^T = exp(K @ Q^T) computed
directly; softmax denominators from ones-columns appended to V.
"""

import os
import sys

import numpy as np

for _p in ("/opt/trn_rl_repo", "/root/.axon_site/_ro/trn_rl_repo"):
    if _p not in sys.path and os.path.isdir(_p):
        sys.path.append(_p)

import concourse.bass as bass
import concourse.mybir as mybir
import concourse.tile as tile
from concourse import bacc
from concourse.bass import ts
from concourse.bass_utils import run_bass_kernel_spmd

F32 = mybir.dt.float32
F16 = mybir.dt.float16
AF = mybir.ActivationFunctionType
ALU = mybir.AluOpType

B, S, D, H = 2, 2048, 1024, 16
DK = D // H            # 64
FF = 4 * D             # 4096
NCORES = 8
HL = H // NCORES       # 2 heads per core
DL = DK * HL           # 128 channels per core
FFL = FF // NCORES     # 512 ff dims per core
KC = D // 128          # 8 contraction chunks of the full model dim
FCL = FFL // 128       # 4 local ff chunks
ST = S // 512          # 4 sequence tiles of 512
SC = S // 128          # 16 sequence chunks of 128
SH = S // 2            # sequence half
SCALE = 1.0 / np.sqrt(DK)
VARF = (S - 1) / S     # unbiased-variance factor applied to 1/var_pop
VW = DK + 1            # 65: per-head stationary width in vO (v + ones col)

RG = [[0, 1, 2, 3, 4, 5, 6, 7]]

_CACHED_NC = None


def _build():
    nc = bacc.Bacc("TRN2", target_bir_lowering=False, debug=False,
                   num_devices=NCORES)

    # ---- per-core external inputs (fp16 except small fp32 biases) ----
    xTd = nc.dram_tensor("xTd", [128, B, KC, S], F16, kind="ExternalInput")
    encd = nc.dram_tensor("encd", [128, B, KC, S], F16, kind="ExternalInput")
    resd = nc.dram_tensor("resd", [128, B, S], F16, kind="ExternalInput")
    wq1d = nc.dram_tensor("wq1d", [128, KC, DL], F16, kind="ExternalInput")
    wk1d = nc.dram_tensor("wk1d", [128, KC, DL], F16, kind="ExternalInput")
    wv1d = nc.dram_tensor("wv1d", [128, KC, DL], F16, kind="ExternalInput")
    wq2d = nc.dram_tensor("wq2d", [128, KC, DL], F16, kind="ExternalInput")
    wk2d = nc.dram_tensor("wk2d", [128, KC, DL], F16, kind="ExternalInput")
    wv2d = nc.dram_tensor("wv2d", [128, KC, DL], F16, kind="ExternalInput")
    w1d = nc.dram_tensor("w1d", [128, KC, FFL], F16, kind="ExternalInput")
    w2d = nc.dram_tensor("w2d", [128, FCL, D], F16, kind="ExternalInput")
    bqkd = nc.dram_tensor("bqkd", [128, 4], F32, kind="ExternalInput")
    # bv values per (head, dk); rows 64:128 duplicate rows 0:64 so the h1
    # stt can use a partition-64-based scalar AP
    bvd = nc.dram_tensor("bvd", [128, 2 * HL], F32, kind="ExternalInput")
    b1d = nc.dram_tensor("b1d", [128, FCL], F32, kind="ExternalInput")
    b2d = nc.dram_tensor("b2d", [128, 1], F32, kind="ExternalInput")

    outT = nc.dram_tensor("outT", [B * DL, S], F32, kind="ExternalOutput")
    DBG = bool(os.environ.get("BASSDBG"))
    if DBG:
        dbg_x1 = nc.dram_tensor("dbg_x1", [B * DL, S], F32,
                                kind="ExternalOutput")
        dbg_x2 = nc.dram_tensor("dbg_x2", [B * DL, S], F32,
                                kind="ExternalOutput")
        dbg_k1 = nc.dram_tensor("dbg_k1", [128, B, S], F16,
                                kind="ExternalOutput")
        dbg_q1 = nc.dram_tensor("dbg_q1", [128, B, ST, 512], F16,
                                kind="ExternalOutput")
        dbg_v1 = nc.dram_tensor("dbg_v1", [128, B, SC, 2 * VW], F16,
                                kind="ExternalOutput")
        dbg_pre = nc.dram_tensor("dbg_pre", [128, B, S], F32,
                                 kind="ExternalOutput")

    def bview(t):   # [B*128, s] -> [p, b, s]
        return t[:].rearrange("(b p) s -> p b s", p=128)

    with tile.TileContext(nc) as tc:
        import contextlib
        ctx = contextlib.ExitStack()
        with ctx:
            sb = ctx.enter_context(tc.tile_pool(name="sb", bufs=1))
            dram = ctx.enter_context(tc.tile_pool(name="dr", bufs=1,
                                                  space="DRAM"))
            pp = ctx.enter_context(tc.tile_pool(name="pp", bufs=2,
                                                space="PSUM"))
            pe = ctx.enter_context(tc.tile_pool(name="pe", bufs=2,
                                                space="PSUM"))
            pz = ctx.enter_context(tc.tile_pool(name="pz", bufs=1,
                                                space="PSUM"))

            # ---- collective bounce buffers (per batch element) ----
            x1b = [dram.tile([DL, S], F16, tag=f"x1b{b}", name=f"x1b{b}")
                   for b in range(B)]
            x1f = [dram.tile([NCORES, DL, S], F16, tag=f"x1f{b}",
                             name=f"x1f{b}", addr_space="Shared")
                   for b in range(B)]
            x2b = [dram.tile([DL, S], F16, tag=f"x2b{b}", name=f"x2b{b}")
                   for b in range(B)]
            x2f = [dram.tile([NCORES, DL, S], F16, tag=f"x2f{b}",
                             name=f"x2f{b}", addr_space="Shared")
                   for b in range(B)]
            # RS per (b, seq-half): few enough that the serial cc stream
            # doesn't congest, small enough that only ~one is tail-exposed
            rsi = [[dram.tile([NCORES, DL, SH], F16, tag=f"rsi{b}{h}",
                              name=f"rsi{b}{h}") for h in range(2)]
                   for b in range(B)]
            rso = [[dram.tile([DL, SH], F16, tag=f"rso{b}{h}",
                              name=f"rso{b}{h}")
                    for h in range(2)] for b in range(B)]
            # dummy collective issued at t=0: the first collective on the
            # device carries the rank-sync barrier (~40-50us of launch skew);
            # firing it up front hides that under the warmup DMAs/compute
            dumb = dram.tile([128, 2], F16, tag="dumb", name="dumb")
            dumf = dram.tile([NCORES, 128, 2], F16, tag="dumf", name="dumf",
                             addr_space="Shared")
            nc.gpsimd.collective_compute(
                "AllGather", ALU.bypass, replica_groups=RG,
                ins=[dumb[:]], outs=[dumf[:]])

            # ---- small persistent tiles ----
            bqk_sb = sb.tile([128, 4], F32, tag="bias", bufs=1)
            nc.sync.dma_start(out=bqk_sb, in_=bqkd[:])
            bv_sb = sb.tile([128, 2 * HL], F32, tag="bias2", bufs=1)
            nc.sync.dma_start(out=bv_sb, in_=bvd[:])
            b1_sb = sb.tile([128, FCL], F32, tag="bias3", bufs=1)
            nc.sync.dma_start(out=b1_sb, in_=b1d[:])
            b2_sb = sb.tile([128, 1], F32, tag="bias4", bufs=1)
            nc.sync.dma_start(out=b2_sb, in_=b2d[:])

            # identity for PE transposes (fp16)
            id16 = sb.tile([128, 128], F16, tag="id16", bufs=1)
            nc.vector.memset(id16, 1.0)
            nc.gpsimd.affine_select(out=id16, in_=id16,
                                    compare_op=ALU.is_equal, fill=0.0,
                                    base=0, channel_multiplier=-1,
                                    pattern=[[1, 128]])

            # ---- weights (all loaded up front; ~3 MB fp16) ----
            def load_w(dram_t, name, cols, tag):
                w = sb.tile([128, KC, cols], F16, tag=tag, bufs=1, name=name)
                nc.sync.dma_start(out=w, in_=dram_t[:])
                return w

            # only sublayer-1 weights now; the rest are loaded after the
            # qkv1 xs loads are queued so the first matmul starts early
            wq1 = load_w(wq1d, "wq1", DL, "wqkv")
            wk1 = load_w(wk1d, "wk1", DL, "wqkv2")
            wv1 = load_w(wv1d, "wv1", DL, "wqkv3")

            # ---- persistent activation tiles ----
            kT1 = sb.tile([128, B, S], F16, tag="kT1", bufs=1, name="kT1")
            kT2 = sb.tile([128, B, S], F16, tag="kT2", bufs=1, name="kT2")
            # vO layout: [h0 ch(64) | ones | h1 ch(64) | ones] => stationary
            # for head h is the contiguous [128, 65] slice at 65*h.
            vO1 = sb.tile([128, B, SC, 2 * VW], F16, tag="vO1", bufs=1,
                          name="vO1")
            vO2 = sb.tile([128, B, SC, 2 * VW], F16, tag="vO2", bufs=1,
                          name="vO2")
            for vO in (vO1, vO2):
                nc.vector.memset(vO[:, :, :, DK:DK + 1], 1.0)
                nc.vector.memset(vO[:, :, :, DK + VW:DK + VW + 1], 1.0)
            qt1 = sb.tile([128, B, ST, 512], F16, tag="qt", bufs=2,
                          name="qt1")
            qt2 = sb.tile([128, B, ST, 512], F16, tag="qt", bufs=2,
                          name="qt2")
            x1 = sb.tile([128, B, S], F32, tag="xl", bufs=2, name="x1")
            x2 = sb.tile([128, B, S], F32, tag="xl", bufs=2, name="x2")

            def proj128(xs, w, bias_col, out_ap):
                """One [128, 512] projection: out = W.T @ x + bias (ACT
                drain psum->sbuf fp16 with per-channel bias)."""
                ps = pp.tile([128, 512], F32, tag="pp", name="ps")
                for k in range(KC):
                    nc.tensor.matmul(ps, w[:, k, :], xs[:, k, :],
                                     start=(k == 0), stop=(k == KC - 1))
                # drain on DVE: ACT is the bottleneck engine in attention
                nc.vector.tensor_scalar(
                    out=out_ap, in0=ps,
                    scalar1=bqk_sb[:, bias_col:bias_col + 1],
                    scalar2=None, op0=ALU.add)

            def load_xs(src_ap):
                xs = sb.tile([128, KC, 512], F16, tag="xs", bufs=4,
                             name="xs")
                nc.sync.dma_start(out=xs, in_=src_ap)
                return xs

            def qkv_tile(xs, b, t, wq, wk, wv, kT, vO, qt, qcol, kcol,
                         bv_off):
                """One (b, 512-seq-tile): q/k transposed projections, v
                projected then flipped back via PE transposes into vO (ones
                columns persist from the initial memset)."""
                if wq is not None:
                    proj128(xs, wq, qcol, qt[:, b, t, :])
                proj128(xs, wk, kcol, kT[:, b, ts(t, 512)])
                # vT [128(2h*dk), 512] (+bias) then transpose per 128-block
                ps = pp.tile([128, 512], F32, tag="pp", name="ps")
                for k in range(KC):
                    nc.tensor.matmul(ps, wv[:, k, :], xs[:, k, :],
                                     start=(k == 0), stop=(k == KC - 1))
                # no bias here: bv is added after softmax-normalize (rows of
                # P sum to 1, so PV(v)+bv == P(v+bv) normalized)
                vt = sb.tile([128, 512], F16, tag="vt", bufs=2, name="vt")
                nc.vector.tensor_copy(out=vt, in_=ps)
                tp = pp.tile([128, 512], F16, tag="pp", name="tp")
                for sc in range(4):
                    nc.tensor.transpose(tp[:, ts(sc, 128)],
                                        vt[:, ts(sc, 128)], id16)
                # one strided copy: [sc, h, dk] -> vO cols {0:64, 65:129}
                nc.vector.tensor_copy(
                    out=vO[:, b, 4 * t:4 * t + 4, :]
                        .rearrange("p c (h w) -> p c h w", h=2)[:, :, :, 0:DK],
                    in_=tp[:, :].rearrange("p (c h w) -> p c h w", c=4, h=2))

            def attn_tile(b, t, qt, kT, vO, xout, resid_ap, bv_off, causal):
                """One (b, sq-tile): per sk-chunk, E^T for both heads in one
                2-bank psum (disjoint PE row groups -> concurrent), exp on
                ACT (fp16 out), causal mask on gpsimd, PV for both heads into
                one 2-bank psum; then one drain + approx-reciprocal denom +
                normalize + bias + residual into xout."""
                nchunks = (4 * t + 4) if causal else SC
                zps = pz.tile([128, 1024], F32, tag="pz", name="zps")
                for c in range(nchunks):
                    eps = pe.tile([128, 1024], F32, tag="pe", name="eps")
                    for h in range(HL):
                        hb = h * 64
                        nc.tensor.matmul(eps[:, ts(h, 512)],
                                         kT[hb:hb + 64, b, ts(c, 128)],
                                         qt[hb:hb + 64, b, t, :],
                                         start=True, stop=True)
                    et = sb.tile([128, 1024], F16, tag="E", bufs=3,
                                 name="et")
                    nc.scalar.activation(out=et, in_=eps, func=AF.Exp,
                                         scale=float(SCALE))
                    if causal and c >= 4 * t:
                        j = c - 4 * t
                        for h in range(HL):
                            nc.gpsimd.affine_select(
                                out=et[:, ts(h, 512)],
                                in_=et[:, ts(h, 512)],
                                compare_op=ALU.is_ge,
                                fill=0.0, base=-(j * 128),
                                channel_multiplier=-1,
                                pattern=[[1, 512]])
                    for h in range(HL):
                        nc.tensor.matmul(
                            zps[0:VW, ts(h, 512)],
                            vO[:, b, c, VW * h:VW * h + VW],
                            et[:, ts(h, 512)],
                            start=(c == 0), stop=(c == nchunks - 1))
                # drain [65, 1024] once; row 64 = softmax denominators
                zsb = sb.tile([VW, 1024], F32, tag="zsb", bufs=2, name="zsb")
                nc.vector.tensor_copy(out=zsb, in_=zps[0:VW, :])
                dr = sb.tile([1, 1024], F32, tag="dr", bufs=2, name="dr")
                nc.vector.tensor_copy(out=dr, in_=zsb[DK:DK + 1, :])
                rb = sb.tile([64, 1024], F32, tag="rb", bufs=2, name="rb")
                nc.gpsimd.partition_broadcast(out_ap=rb, in_ap=dr)
                nc.vector.reciprocal_approx_fast(out=rb, in_=rb)
                zn = sb.tile([64, 1024], F32, tag="zn", bufs=2, name="zn")
                nc.vector.tensor_mul(zn, zsb[0:DK, :], rb)
                # h0 writes in place; h1 needs a partition shift (stt operands
                # must share a start partition; only copies may shift)
                nc.vector.scalar_tensor_tensor(
                    out=xout[0:64, b, ts(t, 512)],
                    in0=zn[:, 0:512],
                    scalar=bv_sb[0:64, bv_off:bv_off + 1], op0=ALU.add,
                    in1=resid_ap[0:64, b, ts(t, 512)], op1=ALU.add)
                nc.vector.tensor_copy(out=xout[64:128, b, ts(t, 512)],
                                      in_=zn[:, 512:1024])
                nc.vector.scalar_tensor_tensor(
                    out=xout[64:128, b, ts(t, 512)],
                    in0=xout[64:128, b, ts(t, 512)],
                    scalar=bv_sb[64:128, bv_off + 1:bv_off + 2], op0=ALU.add,
                    in1=resid_ap[64:128, b, ts(t, 512)], op1=ALU.add)

            def seqnorm_b(xt, b):
                """Sequence-norm of [128, S] f32 (divide by unbiased var)."""
                stats = sb.tile([128, ST, 6], F32, tag="bnst", bufs=2,
                                name="stats")
                for g in range(ST):
                    nc.vector.bn_stats(out=stats[:, g, :],
                                       in_=xt[:, b, ts(g, 512)])
                mv = sb.tile([128, 2], F32, tag="bnmv", bufs=2, name="mv")
                nc.vector.bn_aggr(out=mv, in_=stats)
                r = sb.tile([128, 1], F32, tag="bnr", bufs=2, name="r")
                nc.vector.reciprocal(r, mv[:, 1:2])
                nc.vector.tensor_scalar(out=r, in0=r, scalar1=float(VARF),
                                        scalar2=None, op0=ALU.mult)
                mr = sb.tile([128, 1], F32, tag="bnmr", bufs=2, name="mr")
                nc.vector.scalar_tensor_tensor(
                    out=mr, in0=mv[:, 0:1], scalar=-1.0, op0=ALU.mult,
                    in1=r, op1=ALU.mult)
                nc.vector.scalar_tensor_tensor(
                    out=xt[:, b, :], in0=xt[:, b, :], scalar=r,
                    op0=ALU.mult, in1=mr.to_broadcast((128, S)),
                    op1=ALU.add)

            def ag_issue(xt, b, bb, fb):
                """Cast+store x[:, b, :] to the bounce buffer (SWDGE casting
                DMA on the gpsimd queue) and trigger the AllGather."""
                nc.gpsimd.dma_start(out=bb[:], in_=xt[:, b, :])
                nc.gpsimd.collective_compute(
                    "AllGather", ALU.bypass, replica_groups=RG,
                    ins=[bb[:]], outs=[fb[:]])

            # ================= sublayer 1: causal self-attention ===========
            for b in range(B):
                for t in range(ST):
                    xs = load_xs(xTd.ap()[:, b, :, ts(t, 512)])
                    qkv_tile(xs, b, t, wq1, wk1, wv1, kT1, vO1, qt1,
                             qcol=0, kcol=1, bv_off=0)
            # deferred loads: residual + later-phase weights queue behind
            # the qkv1 xs loads
            resid = sb.tile([128, B, S], F16, tag="res", bufs=1,
                            name="resid")
            nc.sync.dma_start(out=resid, in_=resd[:])
            wq2 = load_w(wq2d, "wq2", DL, "wqkv4")
            wk2 = load_w(wk2d, "wk2", DL, "wqkv5")
            wv2 = load_w(wv2d, "wv2", DL, "wqkv6")
            w1 = sb.tile([128, KC, FFL], F16, tag="w1", bufs=1, name="w1")
            nc.sync.dma_start(out=w1, in_=w1d[:])
            w2 = sb.tile([128, FCL, D], F16, tag="w2", bufs=1, name="w2")
            nc.sync.dma_start(out=w2, in_=w2d[:])
            # prefetch enc tiles now (sync queue, streams during attn1)
            enc_xs = [[load_xs(encd.ap()[:, b, :, ts(t, 512)])
                       for t in range(ST)] for b in range(B)]
            q2_xs = [None] * B
            for b in range(B):
                for t in range(ST):
                    attn_tile(b, t, qt1, kT1, vO1, x1, resid, bv_off=0,
                              causal=True)
                if DBG:
                    nc.sync.dma_start(out=dbg_pre[:, b, :], in_=x1[:, b, :])
                seqnorm_b(x1, b)
                ag_issue(x1, b, x1b[b], x1f[b])
                # q2 loads for this b (gated on AG1(b); behind enc loads in
                # the sync queue, so enc prefetch is never blocked)
                x1f_v = x1f[b][:].rearrange("r p s -> p r s")
                q2_xs[b] = [load_xs(x1f_v[:, :, ts(t, 512)])
                            for t in range(ST)]
                # fill attn1(b)'s ACT-bound lag with enc K/V projections
                for t in range(ST):
                    qkv_tile(enc_xs[b][t], b, t, None, wk2, wv2, kT2, vO2,
                             None, qcol=None, kcol=3, bv_off=HL)
            if DBG:
                for b in range(B):
                    nc.sync.dma_start(out=bview(dbg_x1)[:, b, :],
                                      in_=x1[:, b, :])
                nc.sync.dma_start(out=dbg_k1[:], in_=kT1)
                nc.sync.dma_start(out=dbg_q1[:], in_=qt1)
                nc.sync.dma_start(out=dbg_v1[:], in_=vO1)

            # ================= sublayer 2: cross-attention =================
            # q2 projection interleaved per tile so ACT's exp stream starts
            # after a single projection instead of all eight
            for b in range(B):
                for t in range(ST):
                    proj128(q2_xs[b][t], wq2, 2, qt2[:, b, t, :])
                    attn_tile(b, t, qt2, kT2, vO2, x2, x1, bv_off=HL,
                              causal=False)
                seqnorm_b(x2, b)
                ag_issue(x2, b, x2b[b], x2f[b])
            if DBG:
                for b in range(B):
                    nc.sync.dma_start(out=bview(dbg_x2)[:, b, :],
                                      in_=x2[:, b, :])

            # ================= sublayer 3: FFN =============================
            for b in range(B):
                x2f_v = x2f[b][:].rearrange("r p s -> p r s")
                for t in range(ST):
                    xs = load_xs(x2f_v[:, :, ts(t, 512)])
                    hT = sb.tile([128, FCL, 512], F16, tag="hT", bufs=2,
                                 name="hT")
                    for fc in range(FCL):
                        ps_h = pp.tile([128, 512], F32, tag="pp",
                                       name="ps_h")
                        for k in range(KC):
                            nc.tensor.matmul(ps_h, w1[:, k, ts(fc, 128)],
                                             xs[:, k, :],
                                             start=(k == 0),
                                             stop=(k == KC - 1))
                        # relu(x + b1) on DVE: (in + b1) max 0
                        nc.vector.tensor_scalar(
                            out=hT[:, fc, :], in0=ps_h,
                            scalar1=b1_sb[:, fc:fc + 1], scalar2=0.0,
                            op0=ALU.add, op1=ALU.max)
                    rv = rsi[b][t // 2][:]
                    for ec in range(KC):
                        ps_y = pp.tile([128, 512], F32, tag="pp",
                                       name="ps_y")
                        for fc in range(FCL):
                            nc.tensor.matmul(ps_y, w2[:, fc, ts(ec, 128)],
                                             hT[:, fc, :],
                                             start=(fc == 0),
                                             stop=(fc == FCL - 1))
                        ys = sb.tile([128, 512], F16, tag="ys", bufs=6,
                                     name="ys")
                        nc.vector.tensor_copy(out=ys, in_=ps_y)
                        nc.sync.dma_start(out=rv[ec, :, ts(t % 2, 512)],
                                          in_=ys)
                    if t % 2 == 1:
                        nc.gpsimd.collective_compute(
                            "ReduceScatter", ALU.add, replica_groups=RG,
                            ins=[rsi[b][t // 2][:]],
                            outs=[rso[b][t // 2][:]])

            # ======= y + b2 + x2 residual, seqnorm, write out (per b) ======
            # negative high_priority offset = LOW priority: these ops wait on
            # ReduceScatter results, and the scheduler (whose collective cost
            # model is optimistic) otherwise hoists them into the middle of
            # the FFN's DVE stream, stalling the whole queue on the RS
            ctx.enter_context(tc.high_priority(offset=-1000000))
            x3 = sb.tile([128, B, S], F32, tag="xl", bufs=2, name="x3")
            for b in range(B):
                stats = sb.tile([128, ST, 6], F32, tag="bnst", bufs=2,
                                name="stats")
                for half in range(2):
                    yh = sb.tile([128, SH], F16, tag="yh", bufs=2,
                                 name="yh")
                    nc.sync.dma_start(out=yh, in_=rso[b][half][:])
                    nc.vector.scalar_tensor_tensor(
                        out=x3[:, b, ts(half, SH)], in0=yh,
                        scalar=b2_sb[:, 0:1], op0=ALU.add,
                        in1=x2[:, b, ts(half, SH)], op1=ALU.add)
                    for g in range(2):
                        nc.vector.bn_stats(
                            out=stats[:, 2 * half + g, :],
                            in_=x3[:, b, ts(2 * half + g, 512)])
                mv = sb.tile([128, 2], F32, tag="bnmv", bufs=2, name="mv")
                nc.vector.bn_aggr(out=mv, in_=stats)
                r = sb.tile([128, 1], F32, tag="bnr", bufs=2, name="r")
                nc.vector.reciprocal(r, mv[:, 1:2])
                nc.vector.tensor_scalar(out=r, in0=r, scalar1=float(VARF),
                                        scalar2=None, op0=ALU.mult)
                mr = sb.tile([128, 1], F32, tag="bnmr", bufs=2, name="mr")
                nc.vector.scalar_tensor_tensor(
                    out=mr, in0=mv[:, 0:1], scalar=-1.0, op0=ALU.mult,
                    in1=r, op1=ALU.mult)
                nc.vector.scalar_tensor_tensor(
                    out=x3[:, b, :], in0=x3[:, b, :], scalar=r,
                    op0=ALU.mult, in1=mr.to_broadcast((128, S)),
                    op1=ALU.add)
                nc.sync.dma_start(out=bview(outT)[:, b, :], in_=x3[:, b, :])

    nc.compile()
    return nc


def _get_nc():
    global _CACHED_NC
    if _CACHED_NC is None:
        _CACHED_NC = _build()
    return _CACHED_NC


def _chunked(a):
    """[D, N] -> [128, D//128, N] with [p, c, n] = a[128c+p, n]."""
    d, n = a.shape
    return np.ascontiguousarray(
        a.reshape(d // 128, 128, n).transpose(1, 0, 2).astype(np.float16))


def _make_in_maps(decoder_input, encode_input,
                  Wq1, Wk1, Wv1, bq1, bk1, bv1,
                  Wq2, Wk2, Wv2, bq2, bk2, bv2,
                  W1, b1, W2, b2):
    xT = np.ascontiguousarray(
        np.transpose(np.asarray(decoder_input, np.float32), (0, 2, 1)))
    eT = np.transpose(np.asarray(encode_input, np.float32), (0, 2, 1))
    # [128, B, KC, S] fp16
    xTd_all = np.ascontiguousarray(
        xT.reshape(B, KC, 128, S).transpose(2, 0, 1, 3).astype(np.float16))
    encd_all = np.ascontiguousarray(
        eT.reshape(B, KC, 128, S).transpose(2, 0, 1, 3).astype(np.float16))
    in_maps = []
    for r in range(NCORES):
        hs = slice(DL * r, DL * (r + 1))
        fs = slice(FFL * r, FFL * (r + 1))
        resd = np.ascontiguousarray(
            xT[:, hs, :].transpose(1, 0, 2).astype(np.float16))
        bqk_arr = np.stack([bq1[hs], bk1[hs], bq2[hs], bk2[hs]],
                           axis=1).astype(np.float32)  # [128, 4]
        bv_arr = np.concatenate([
            bv1[hs].reshape(HL, DK).T, bv2[hs].reshape(HL, DK).T,
        ], axis=1).astype(np.float32)                  # [64, 4]
        bv_arr = np.concatenate([bv_arr, bv_arr], axis=0)  # [128, 4]
        in_maps.append({
            "xTd": xTd_all,
            "encd": encd_all,
            "resd": resd,
            "wq1d": _chunked(np.ascontiguousarray(Wq1[:, hs])),
            "wk1d": _chunked(np.ascontiguousarray(Wk1[:, hs])),
            "wv1d": _chunked(np.ascontiguousarray(Wv1[:, hs])),
            "wq2d": _chunked(np.ascontiguousarray(Wq2[:, hs])),
            "wk2d": _chunked(np.ascontiguousarray(Wk2[:, hs])),
            "wv2d": _chunked(np.ascontiguousarray(Wv2[:, hs])),
            "w1d": _chunked(np.ascontiguousarray(W1[:, fs])),
            "w2d": _chunked(np.ascontiguousarray(W2[fs, :])),
            "bqkd": bqk_arr,
            "bvd": bv_arr,
            "b1d": np.ascontiguousarray(
                b1[fs].reshape(FCL, 128).T.astype(np.float32)),
            "b2d": np.ascontiguousarray(
                b2[hs].reshape(128, 1).astype(np.float32)),
        })
    return in_maps


def kernel(**inputs):
    nc = _get_nc()
    in_maps = _make_in_maps(**{k: np.asarray(v) for k, v in inputs.items()})
    res = run_bass_kernel_spmd(nc, in_maps, core_ids=list(range(NCORES)),
                               trace=False)
    out = np.empty((B, S, D), np.float32)
    for r in range(NCORES):
        hs = slice(DL * r, DL * (r + 1))
        o = res.results[r]["outT"]                     # [B*DL, S]
        for b in range(B):
            out[b, :, hs] = o[b * DL:(b + 1) * DL].T
    return out
